# revision 15
# baseline (speedup 1.0000x reference)
"""Multi-head attention (B=2, S=2048, D=1024, H=16) on 8 Trainium2 NeuronCores.

Sharding: core c handles batch b = c//4 and head group g = c%4 (4 heads, 256
model dims).  Each core computes q/k/v projections for its heads, attention,
and a partial output projection (row-parallel over its 256 head dims); the
host sums the 4 partials per batch and adds the bias.

All activations live in transposed layouts so the tensor engine contraction
dim always sits on SBUF partitions:
  xT  [d, s]  (host pre-transposed)
  qT/kT [e, s] per head-pair tile (heads at partitions 0-63 / 64-127)
  v   [s, e+1] per (s-block, head) with a trailing ones column so the PV
      matmul's 65th output row is the softmax denominator.
Scores are computed transposed [ks, qs]; softmax needs no max-subtraction
(scores ~ N(0,1)) and no free-dim reduction: exp runs elementwise on ACT and
the denominator rides the PV matmul.  Normalization broadcasts 1/denominator
via a DRAM round-trip (partition-broadcast DMA) and one DVE multiply.

Matmuls use float32r (~1.5e-4 rel err, full PE rate for free dims >= 256).
"""

import os
import sys

import numpy as np

for _p in ("/opt/trn_rl_repo", "/root/.axon_site/_ro/trn_rl_repo"):
    if os.path.isdir(_p) and _p not in sys.path:
        sys.path.insert(0, _p)

import bass_rust
import concourse.bass as bass
import concourse.mybir as mybir
import concourse.tile as tile
from concourse.bass_utils import run_bass_kernel_spmd
from concourse.vector_clock import ScopedClock, VectorClock
from contextlib import ExitStack

F32 = mybir.dt.float32
F32R = mybir.dt.float32r
EXP = mybir.ActivationFunctionType.Exp

B = 2
S = 2048
D = 1024
H = 16
HD = 64
NCORES = 8
GROUPS = 4          # head groups (cores per batch)
HG = H // GROUPS    # heads per core = 4
E = HG * HD         # head dims per core = 256
KT = D // 128       # contraction tiles over model dim = 8
SB = S // 128       # s blocks = 16
QB = S // 512       # 512-wide qs blocks = 4

_carrier_counter = [0]


def _split_multi_waits(ordered):
    """This walrus build allows one sync wait per instruction; Tile's wait
    assignment can attach several.  Hoist extras onto same-engine InstNoOp
    carriers placed immediately before the instruction."""
    for bb_name, insts in ordered.items():
        new_list = []
        for inst in insts:
            si = inst.sync_info
            waits = list(si.on_wait) if si is not None else []
            if len(waits) > 1:
                for w in waits[:-1]:
                    _carrier_counter[0] += 1
                    carrier = mybir.InstNoOp(
                        name=f"I-waitc-{_carrier_counter[0]}", ins=[], outs=[]
                    )
                    carrier.engine = inst.engine
                    carrier.sync_info = bass_rust.SyncInfo(on_wait=[w], on_update=[])
                    new_list.append(carrier)
                inst.sync_info = bass_rust.SyncInfo(
                    on_wait=[waits[-1]],
                    on_update=list(si.on_update) if si is not None else [],
                )
            new_list.append(inst)
        ordered[bb_name] = new_list


class _TileContext(tile.TileContext):
    """TileContext adapted to the one-sync-wait-per-instruction walrus."""

    def _lower_ordered_insts(self, ordered):
        _split_multi_waits(ordered)
        return super()._lower_ordered_insts(ordered)

    def _drain_and_barrier(self, tick_clock, wait_clock):
        gc = tick_clock.global_clock
        for proc in range(len(gc)):
            if gc[proc] <= 0:
                continue
            cur = VectorClock([0 if i == proc else gc[i] for i in range(len(gc))])
            nop = self.nc.sync.nop()
            wait_clock.add_sem_waits(
                nop.ins, ScopedClock({None: gc}), ScopedClock({None: cur})
            )
        drain_inst = self.nc.sync.drain()
        wait_clock.add_sem_waits(
            drain_inst.ins, ScopedClock({None: gc}), ScopedClock({None: gc.copy()})
        )
        self.nc.all_engine_barrier()
        assert self.sems is not None
        popped = self.nc._tile_sem_poison_stack.pop()
        assert popped is self._sem_poison
        self.nc.clear_and_free_semaphores(list(self.sems.allocated().values()))
        self.nc.all_engine_barrier()


def build_nc(reps=1):
    nc = bass.Bass()
    xT = nc.declare_dram_parameter("xT", [D, S], F32R, isOutput=False)
    wqT = nc.declare_dram_parameter("wqT", [D, E], F32R, isOutput=False)
    wkT = nc.declare_dram_parameter("wkT", [D, E], F32R, isOutput=False)
    wvT = nc.declare_dram_parameter("wvT", [D, E], F32R, isOutput=False)
    woT = nc.declare_dram_parameter("woT", [E, D], F32R, isOutput=False)
    out = nc.declare_dram_parameter("out_partial", [S, D], F32, isOutput=True)
    ones_d = nc.declare_dram_parameter("ones_d", [128, SB * HG], F32R, isOutput=False)
    den_d = nc.dram_tensor("den_scratch", [HG, S], F32)

    with _TileContext(nc) as tc, ExitStack() as outer:
      for _rep in range(reps):
        ctx = outer.enter_context(ExitStack())
        # ---- persistent activation tiles (live across phases) ----
        act_pool = ctx.enter_context(tc.tile_pool(name="acts", bufs=1))
        qT_sb = [act_pool.tile([128, S], F32R, tag=f"qT{m}", name=f"qT{m}") for m in range(2)]
        kT_sb = [act_pool.tile([128, S], F32R, tag=f"kT{m}", name=f"kT{m}") for m in range(2)]
        v_sb = act_pool.tile([128, SB, HG, HD + 1], F32R, tag="v")
        wo_sb = [act_pool.tile([64, D], F32R, tag=f"wo{h}", name=f"wo{h}") for h in range(HG)]

        # ---- phase 1: load inputs, project q/k/v ----
        with ExitStack() as c1:
            in_pool = c1.enter_context(tc.tile_pool(name="ins", bufs=1))
            ps_qk = c1.enter_context(tc.tile_pool(name="ps_qk", bufs=2, space="PSUM"))
            ps_v = c1.enter_context(tc.tile_pool(name="ps_v", bufs=2, space="PSUM"))

            x_sb = in_pool.tile([128, KT, S], F32R, tag="x")
            wq_sb = in_pool.tile([128, KT, E], F32R, tag="wq")
            wk_sb = in_pool.tile([128, KT, E], F32R, tag="wk")
            wv_sb = in_pool.tile([128, KT, E], F32R, tag="wv")
            for k in range(KT):
                nc.sync.dma_start(x_sb[:, k, :], xT[k * 128:(k + 1) * 128, :])
                nc.sync.dma_start(wq_sb[:, k, :], wqT[k * 128:(k + 1) * 128, :])
                nc.sync.dma_start(wk_sb[:, k, :], wkT[k * 128:(k + 1) * 128, :])
                nc.sync.dma_start(wv_sb[:, k, :], wvT[k * 128:(k + 1) * 128, :])
            for h in range(HG):
                nc.sync.dma_start(wo_sb[h][:, :], woT[h * 64:(h + 1) * 64, :])

            # ones column for the softmax-denominator rows of v
            nc.sync.dma_start(
                v_sb[:, :, :, HD],
                ones_d[:, :].rearrange("p (s h) -> p s h", s=SB),
            )

            # qT / kT: W-stationary, out [e(128), s]
            for w_sb, dst in ((wq_sb, qT_sb), (wk_sb, kT_sb)):
                for m in range(2):
                    for nb in range(QB):
                        ps = ps_qk.tile([128, 512], F32)
                        for k in range(KT):
                            nc.tensor.matmul(
                                ps[:],
                                w_sb[:, k, m * 128:(m + 1) * 128],
                                x_sb[:, k, nb * 512:(nb + 1) * 512],
                                start=(k == 0),
                                stop=(k == KT - 1),
                            )
                        nc.vector.tensor_copy(
                            dst[m][:, nb * 512:(nb + 1) * 512], ps[:]
                        )
            # v: x-stationary, out [s(128), e]
            for sb in range(SB):
                ps = ps_v.tile([128, E], F32)
                for k in range(KT):
                    nc.tensor.matmul(
                        ps[:],
                        x_sb[:, k, sb * 128:(sb + 1) * 128],
                        wv_sb[:, k, :],
                        start=(k == 0),
                        stop=(k == KT - 1),
                    )
                nc.vector.tensor_copy(
                    v_sb[:, sb, :, 0:HD],
                    ps[:].rearrange("p (h e) -> p h e", h=HG),
                )

        # ---- phase 2: attention per head-pair (m), per qs-half ----
        attn_pool = ctx.enter_context(tc.tile_pool(name="attn", bufs=1))
        attn_sb = [attn_pool.tile([64, S], F32R, tag=f"at{h}", name=f"at{h}") for h in range(HG)]
        with ExitStack() as c2:
            p_pool = c2.enter_context(tc.tile_pool(name="pexp", bufs=3))
            rb_pool = c2.enter_context(tc.tile_pool(name="rbc", bufs=2))
            den_pool = c2.enter_context(tc.tile_pool(name="den", bufs=2))
            ps_pv = c2.enter_context(tc.tile_pool(name="ps_pv", bufs=1, space="PSUM"))
            ps_sc = c2.enter_context(tc.tile_pool(name="ps_sc", bufs=1, space="PSUM"))

            for m in range(2):
                for qh in range(2):  # qs halves of 1024
                    out_ps = [
                        [ps_pv.tile([128, 512], F32, tag=f"pv{r}{qq}", name=f"pv{r}{qq}") for qq in range(2)]
                        for r in range(2)
                    ]
                    for ksb in range(SB):
                        sc = [ps_sc.tile([128, 1024], F32, tag=f"sc{r}", name=f"sc{r}") for r in range(2)]
                        for qq in range(2):
                            for r in range(2):
                                nc.tensor.matmul(
                                    sc[r][:, qq * 512:(qq + 1) * 512],
                                    kT_sb[m][64 * r:64 * r + 64,
                                             ksb * 128:(ksb + 1) * 128],
                                    qT_sb[m][64 * r:64 * r + 64,
                                             qh * 1024 + qq * 512:
                                             qh * 1024 + (qq + 1) * 512],
                                    start=True,
                                    stop=True,
                                )
                        pt = [None, None]
                        for r in range(2):
                            pt[r] = p_pool.tile([128, 1024], F32R, tag="p", name=f"p{r}")
                            nc.scalar.activation(pt[r][:], sc[r][:], EXP)
                        for r in range(2):
                            for qq in range(2):
                                nc.tensor.matmul(
                                    out_ps[r][qq][0:HD + 1, :],
                                    v_sb[:, ksb, 2 * m + r, :],
                                    pt[r][:, qq * 512:(qq + 1) * 512],
                                    start=(ksb == 0),
                                    stop=(ksb == SB - 1),
                                )
                    # normalize: attn = attnU * (1/denom), denom = row 64
                    for r in range(2):
                        h = 2 * m + r
                        den = den_pool.tile([128, 1024], F32, tag="den")
                        for qq in range(2):
                            nc.vector.tensor_copy(
                                den[64:65, qq * 512:(qq + 1) * 512],
                                out_ps[r][qq][64:65, :],
                            )
                        nc.vector.reciprocal(den[64:65, :], den[64:65, :])
                        nc.sync.dma_start(
                            den_d[h:h + 1, qh * 1024:(qh + 1) * 1024], den[64:65, :]
                        )
                        rb = rb_pool.tile([64, 1024], F32, tag="rb")
                        nc.sync.dma_start(
                            rb[:, :],
                            den_d[h:h + 1, qh * 1024:(qh + 1) * 1024]
                            .to_broadcast((64, 1024)),
                        )
                        for qq in range(2):
                            qs0 = qh * 1024 + qq * 512
                            nc.vector.tensor_mul(
                                attn_sb[h][:, qs0:qs0 + 512],
                                out_ps[r][qq][0:64, :],
                                rb[:, qq * 512:(qq + 1) * 512],
                            )

        # ---- phase 3: output projection (row-parallel partial) ----
        with ExitStack() as c3:
            stage_pool = c3.enter_context(tc.tile_pool(name="ostage", bufs=3))
            ps_o = c3.enter_context(
                tc.tile_pool(name="ps_o", bufs=2, space="PSUM")
            )
            for sb in range(SB):
                for nb in range(2):
                    ps = ps_o.tile([128, 512], F32)
                    for h in range(HG):
                        nc.tensor.matmul(
                            ps[:],
                            attn_sb[h][:, sb * 128:(sb + 1) * 128],
                            wo_sb[h][:, nb * 512:(nb + 1) * 512],
                            start=(h == 0),
                            stop=(h == HG - 1),
                        )
                    st = stage_pool.tile([128, 512], F32, tag="st")
                    nc.vector.tensor_copy(st[:], ps[:])
                    nc.sync.dma_start(
                        out[sb * 128:(sb + 1) * 128, nb * 512:(nb + 1) * 512],
                        st[:],
                    )
        ctx.close()
    return nc


_NC_CACHE = None


def _get_nc():
    global _NC_CACHE
    if _NC_CACHE is None:
        _NC_CACHE = build_nc()
    return _NC_CACHE


_EXEC_CACHE = None


def _get_executor():
    """Build + jit the SPMD executable once; reuse across kernel() calls.

    Mirrors concourse.bass2jax.run_bass_via_pjrt, which re-jits on every
    call (full retrace + executable reload); caching shaves seconds/call."""
    global _EXEC_CACHE
    if _EXEC_CACHE is not None:
        return _EXEC_CACHE
    import jax
    from jax.sharding import Mesh, PartitionSpec
    from jax.experimental.shard_map import shard_map
    from concourse import bass2jax as b2j

    nc = _get_nc()
    b2j.install_neuronx_cc_hook()
    assert nc.dbg_addr is None
    partition_name = (
        nc.partition_id_tensor.name if nc.partition_id_tensor is not None else None
    )

    in_names, out_names, out_avals = [], [], []
    for alloc in nc.m.functions[0].allocations:
        if not isinstance(alloc, mybir.MemoryLocationSet):
            continue
        name = alloc.memorylocations[0].name
        if alloc.kind == "ExternalInput":
            if name != partition_name:
                in_names.append(name)
        elif alloc.kind == "ExternalOutput":
            out_names.append(name)
            out_avals.append(
                jax.core.ShapedArray(
                    tuple(alloc.tensor_shape), mybir.dt.np(alloc.dtype)
                )
            )
    n_params = len(in_names)
    n_outs = len(out_avals)
    all_names = in_names + out_names
    if partition_name is not None:
        all_names = all_names + [partition_name]

    def _body(*args):
        operands = list(args)
        if partition_name is not None:
            operands.append(b2j.partition_id_tensor())
        outs = b2j._bass_exec_p.bind(
            *operands,
            out_avals=tuple(out_avals),
            in_names=tuple(all_names),
            out_names=tuple(out_names),
            lowering_input_output_aliases=(),
            sim_require_finite=True,
            sim_require_nnan=True,
            nc=nc,
        )
        return tuple(outs)

    devices = jax.devices()[:NCORES]
    mesh = Mesh(np.asarray(devices), ("core",))
    donate = tuple(range(n_params, n_params + n_outs))
    sharded = jax.jit(
        shard_map(
            _body,
            mesh=mesh,
            in_specs=(PartitionSpec("core"),) * (n_params + n_outs),
            out_specs=(PartitionSpec("core"),) * n_outs,
            check_rep=False,
        ),
        donate_argnums=donate,
        keep_unused=True,
    )
    _EXEC_CACHE = {
        "sharded": sharded,
        "in_names": in_names,
        "out_names": out_names,
        "out_avals": out_avals,
    }
    return _EXEC_CACHE


def _run_spmd(in_maps):
    ex = _get_executor()
    concat_in = [
        np.concatenate([np.asarray(m[name]) for m in in_maps], axis=0)
        for name in ex["in_names"]
    ]
    concat_zeros = [
        np.zeros((NCORES * a.shape[0], *a.shape[1:]), a.dtype)
        for a in ex["out_avals"]
    ]
    out_arrs = ex["sharded"](*concat_in, *concat_zeros)
    results = []
    for c in range(NCORES):
        results.append({
            name: np.asarray(out_arrs[i]).reshape(
                NCORES, *ex["out_avals"][i].shape
            )[c]
            for i, name in enumerate(ex["out_names"])
        })
    return results


def _shard_inputs(x, Wq, Wk, Wv, Wo):
    scale = np.float32(1.0 / np.sqrt(HD))
    global _ONES
    _ONES = np.ones((128, SB * HG), dtype=np.float32)
    in_maps = []
    xT_b = [np.ascontiguousarray(x[b].T) for b in range(B)]
    for c in range(NCORES):
        b, g = divmod(c, GROUPS)
        sl = slice(g * E, (g + 1) * E)
        in_maps.append({
            "ones_d": _ONES,
            "xT": xT_b[b],
            "wqT": np.ascontiguousarray(Wq[sl, :].T * scale),
            "wkT": np.ascontiguousarray(Wk[sl, :].T),
            "wvT": np.ascontiguousarray(Wv[sl, :].T),
            "woT": np.ascontiguousarray(Wo[:, sl].T),
        })
    return in_maps


def kernel(x, Wq, Wk, Wv, Wo, bo):
    x = np.asarray(x, dtype=np.float32)
    in_maps = _shard_inputs(
        x,
        np.asarray(Wq, dtype=np.float32),
        np.asarray(Wk, dtype=np.float32),
        np.asarray(Wv, dtype=np.float32),
        np.asarray(Wo, dtype=np.float32),
    )
    results = _run_spmd(in_maps)
    bo = np.asarray(bo, dtype=np.float32)
    out = np.empty((B, S, D), dtype=np.float32)
    for b in range(B):
        acc = np.zeros((S, D), dtype=np.float64)
        for g in range(GROUPS):
            acc += results[b * GROUPS + g]["out_partial"]
        out[b] = (acc + bo.astype(np.float64)).astype(np.float32)
    return out


# revision 24
# speedup vs baseline: 1.2287x; 1.2287x over previous
"""Multi-head attention (B=2, S=2048, D=1024, H=16) on 8 Trainium2 NeuronCores.

Sharding: core c handles batch b = c//4 and head group g = c%4 (4 heads, 256
model dims).  Each core computes q/k/v projections for its heads, attention,
and a partial output projection (row-parallel over its 256 head dims); the
host sums the 4 partials per batch and adds the bias.

All activations live in transposed layouts so the tensor engine contraction
dim always sits on SBUF partitions:
  xT  [d, s]  (host pre-transposed)
  qT/kT [e, s] per head-pair tile (heads at partitions 0-63 / 64-127)
  v   [s, e+1] per (s-block, head) with a trailing ones column so the PV
      matmul's 65th output row is the softmax denominator.
Scores are computed transposed [ks, qs]; softmax needs no max-subtraction
(scores ~ N(0,1)) and no free-dim reduction: exp runs elementwise on ACT and
the denominator rides the PV matmul.  Normalization broadcasts 1/denominator
via a DRAM round-trip (partition-broadcast DMA) and one DVE multiply.

Matmuls use float32r (~1.5e-4 rel err, full PE rate for free dims >= 256).
"""

import os
import sys

import numpy as np

for _p in ("/opt/trn_rl_repo", "/root/.axon_site/_ro/trn_rl_repo"):
    if os.path.isdir(_p) and _p not in sys.path:
        sys.path.insert(0, _p)

import bass_rust
import concourse.bass as bass
import concourse.mybir as mybir
import concourse.tile as tile
from concourse.bass_utils import run_bass_kernel_spmd
from concourse.vector_clock import ScopedClock, VectorClock
from contextlib import ExitStack

F32 = mybir.dt.float32
F32R = mybir.dt.float32r
EXP = mybir.ActivationFunctionType.Exp

B = 2
S = 2048
D = 1024
H = 16
HD = 64
NCORES = 8
GROUPS = 4          # head groups (cores per batch)
HG = H // GROUPS    # heads per core = 4
E = HG * HD         # head dims per core = 256
KT = D // 128       # contraction tiles over model dim = 8
SB = S // 128       # s blocks = 16
QB = S // 512       # 512-wide qs blocks = 4

_carrier_counter = [0]


def _split_multi_waits(ordered):
    """This walrus build allows one sync wait per instruction; Tile's wait
    assignment can attach several.  Hoist extras onto same-engine InstNoOp
    carriers placed immediately before the instruction."""
    for bb_name, insts in ordered.items():
        new_list = []
        for inst in insts:
            si = inst.sync_info
            waits = list(si.on_wait) if si is not None else []
            if len(waits) > 1:
                for w in waits[:-1]:
                    _carrier_counter[0] += 1
                    carrier = mybir.InstNoOp(
                        name=f"I-waitc-{_carrier_counter[0]}", ins=[], outs=[]
                    )
                    carrier.engine = inst.engine
                    carrier.sync_info = bass_rust.SyncInfo(on_wait=[w], on_update=[])
                    new_list.append(carrier)
                inst.sync_info = bass_rust.SyncInfo(
                    on_wait=[waits[-1]],
                    on_update=list(si.on_update) if si is not None else [],
                )
            new_list.append(inst)
        ordered[bb_name] = new_list


class _TileContext(tile.TileContext):
    """TileContext adapted to the one-sync-wait-per-instruction walrus."""

    def _lower_ordered_insts(self, ordered):
        _split_multi_waits(ordered)
        return super()._lower_ordered_insts(ordered)

    def _drain_and_barrier(self, tick_clock, wait_clock):
        gc = tick_clock.global_clock
        for proc in range(len(gc)):
            if gc[proc] <= 0:
                continue
            cur = VectorClock([0 if i == proc else gc[i] for i in range(len(gc))])
            nop = self.nc.sync.nop()
            wait_clock.add_sem_waits(
                nop.ins, ScopedClock({None: gc}), ScopedClock({None: cur})
            )
        drain_inst = self.nc.sync.drain()
        wait_clock.add_sem_waits(
            drain_inst.ins, ScopedClock({None: gc}), ScopedClock({None: gc.copy()})
        )
        self.nc.all_engine_barrier()
        assert self.sems is not None
        popped = self.nc._tile_sem_poison_stack.pop()
        assert popped is self._sem_poison
        self.nc.clear_and_free_semaphores(list(self.sems.allocated().values()))
        self.nc.all_engine_barrier()


def build_nc(reps=1):
    nc = bass.Bass()
    xT = nc.declare_dram_parameter("xT", [D, S], F32R, isOutput=False)
    wqT = nc.declare_dram_parameter("wqT", [D, E], F32R, isOutput=False)
    wkT = nc.declare_dram_parameter("wkT", [D, E], F32R, isOutput=False)
    wvT = nc.declare_dram_parameter("wvT", [D, E], F32R, isOutput=False)
    woT = nc.declare_dram_parameter("woT", [E, D], F32R, isOutput=False)
    out = nc.declare_dram_parameter("out_partial", [S, D], F32, isOutput=True)
    ones_d = nc.declare_dram_parameter("ones_d", [128, SB * HG], F32R, isOutput=False)
    den_d = nc.dram_tensor("den_scratch", [HG, S], F32)

    with _TileContext(nc) as tc, ExitStack() as outer:
      for _rep in range(reps):
        ctx = outer.enter_context(ExitStack())
        # ---- persistent activation tiles (live across phases) ----
        act_pool = ctx.enter_context(tc.tile_pool(name="acts", bufs=1))
        qT_sb = [act_pool.tile([128, S], F32R, tag=f"qT{m}", name=f"qT{m}") for m in range(2)]
        kT_sb = [act_pool.tile([128, S], F32R, tag=f"kT{m}", name=f"kT{m}") for m in range(2)]
        v_sb = act_pool.tile([128, SB, HG, HD + 1], F32R, tag="v")
        wo_sb = [act_pool.tile([64, D], F32R, tag=f"wo{h}", name=f"wo{h}") for h in range(HG)]

        # ---- phase 1: load inputs, project q/k/v ----
        with ExitStack() as c1:
            in_pool = c1.enter_context(tc.tile_pool(name="ins", bufs=1))
            ps_qk = c1.enter_context(tc.tile_pool(name="ps_qk", bufs=2, space="PSUM"))
            ps_v = c1.enter_context(tc.tile_pool(name="ps_v", bufs=2, space="PSUM"))

            x_sb = in_pool.tile([128, KT, S], F32R, tag="x")
            wq_sb = in_pool.tile([128, KT, E], F32R, tag="wq")
            wk_sb = in_pool.tile([128, KT, E], F32R, tag="wk")
            wv_sb = in_pool.tile([128, KT, E], F32R, tag="wv")
            # split loads across the two HWDGE queues (SP + ACT) so the
            # phase-1 matmul ramp isn't paced by one DMA queue
            for k in range(KT):
                eng = nc.sync if k % 2 == 0 else nc.gpsimd
                eng2 = nc.gpsimd if k % 2 == 0 else nc.sync
                eng.dma_start(x_sb[:, k, :], xT[k * 128:(k + 1) * 128, :])
                eng2.dma_start(wq_sb[:, k, :], wqT[k * 128:(k + 1) * 128, :])
                eng.dma_start(wk_sb[:, k, :], wkT[k * 128:(k + 1) * 128, :])
                eng2.dma_start(wv_sb[:, k, :], wvT[k * 128:(k + 1) * 128, :])
            for h in range(HG):
                nc.gpsimd.dma_start(wo_sb[h][:, :], woT[h * 64:(h + 1) * 64, :])

            # ones column for the softmax-denominator rows of v
            nc.sync.dma_start(
                v_sb[:, :, :, HD],
                ones_d[:, :].rearrange("p (s h) -> p s h", s=SB),
            )

            # qT / kT: W-stationary, out [e(128), s]
            for w_sb, dst in ((wq_sb, qT_sb), (wk_sb, kT_sb)):
                for m in range(2):
                    for nb in range(QB):
                        ps = ps_qk.tile([128, 512], F32)
                        for k in range(KT):
                            nc.tensor.matmul(
                                ps[:],
                                w_sb[:, k, m * 128:(m + 1) * 128],
                                x_sb[:, k, nb * 512:(nb + 1) * 512],
                                start=(k == 0),
                                stop=(k == KT - 1),
                            )
                        # alternate copies between DVE and ACT (ACT idle here)
                        if nb % 2 == 0:
                            nc.vector.tensor_copy(
                                dst[m][:, nb * 512:(nb + 1) * 512], ps[:]
                            )
                        else:
                            nc.scalar.copy(
                                dst[m][:, nb * 512:(nb + 1) * 512], ps[:]
                            )
            # v: x-stationary, out [s(128), e]
            for sb in range(SB):
                ps = ps_v.tile([128, E], F32)
                for k in range(KT):
                    nc.tensor.matmul(
                        ps[:],
                        x_sb[:, k, sb * 128:(sb + 1) * 128],
                        wv_sb[:, k, :],
                        start=(k == 0),
                        stop=(k == KT - 1),
                    )
                if sb % 2 == 0:
                    nc.vector.tensor_copy(
                        v_sb[:, sb, :, 0:HD],
                        ps[:].rearrange("p (h e) -> p h e", h=HG),
                    )
                else:
                    nc.scalar.copy(
                        v_sb[:, sb, :, 0:HD],
                        ps[:].rearrange("p (h e) -> p h e", h=HG),
                    )

        # ---- phase 2: attention per head-pair (m), per qs-half ----
        attn_pool = ctx.enter_context(tc.tile_pool(name="attn", bufs=1))
        attn_sb = [attn_pool.tile([64, S], F32R, tag=f"at{h}", name=f"at{h}") for h in range(HG)]
        with ExitStack() as c2:
            p_pool = c2.enter_context(tc.tile_pool(name="pexp", bufs=4))
            rb_pool = c2.enter_context(tc.tile_pool(name="rbc", bufs=2))
            den_pool = c2.enter_context(tc.tile_pool(name="den", bufs=2))
            ps_pv = c2.enter_context(tc.tile_pool(name="ps_pv", bufs=1, space="PSUM"))
            ps_sc = c2.enter_context(tc.tile_pool(name="ps_sc", bufs=1, space="PSUM"))

            for m in range(2):
                for qh in range(2):  # qs halves of 1024
                    out_ps = [
                        [ps_pv.tile([128, 512], F32, tag=f"pv{r}{qq}", name=f"pv{r}{qq}") for qq in range(2)]
                        for r in range(2)
                    ]
                    for ksb in range(SB):
                        sc = [ps_sc.tile([128, 1024], F32, tag=f"sc{r}", name=f"sc{r}") for r in range(2)]
                        for qq in range(2):
                            for r in range(2):
                                nc.tensor.matmul(
                                    sc[r][:, qq * 512:(qq + 1) * 512],
                                    kT_sb[m][64 * r:64 * r + 64,
                                             ksb * 128:(ksb + 1) * 128],
                                    qT_sb[m][64 * r:64 * r + 64,
                                             qh * 1024 + qq * 512:
                                             qh * 1024 + (qq + 1) * 512],
                                    start=True,
                                    stop=True,
                                )
                        pt = [None, None]
                        for r in range(2):
                            pt[r] = p_pool.tile([128, 1024], F32R, tag="p", name=f"p{r}")
                            nc.scalar.activation(pt[r][:], sc[r][:], EXP)
                        for r in range(2):
                            for qq in range(2):
                                nc.tensor.matmul(
                                    out_ps[r][qq][0:HD + 1, :],
                                    v_sb[:, ksb, 2 * m + r, :],
                                    pt[r][:, qq * 512:(qq + 1) * 512],
                                    start=(ksb == 0),
                                    stop=(ksb == SB - 1),
                                )
                    # normalize: attn = attnU * (1/denom), denom = row 64.
                    # Copy attnU + denom out of PSUM immediately (frees the
                    # accumulation banks for the next group); the broadcasted
                    # reciprocal multiply happens later in SBUF, off the
                    # PSUM critical path.
                    for r in range(2):
                        h = 2 * m + r
                        den = den_pool.tile([128, 1024], F32, tag="den")
                        for qq in range(2):
                            qs0 = qh * 1024 + qq * 512
                            nc.vector.tensor_copy(
                                den[64:65, qq * 512:(qq + 1) * 512],
                                out_ps[r][qq][64:65, :],
                            )
                            nc.vector.tensor_copy(
                                attn_sb[h][:, qs0:qs0 + 512],
                                out_ps[r][qq][0:64, :],
                            )
                        nc.vector.reciprocal(den[64:65, :], den[64:65, :])
                        nc.sync.dma_start(
                            den_d[h:h + 1, qh * 1024:(qh + 1) * 1024], den[64:65, :]
                        )
                        rb = rb_pool.tile([64, 1024], F32, tag="rb")
                        nc.sync.dma_start(
                            rb[:, :],
                            den_d[h:h + 1, qh * 1024:(qh + 1) * 1024]
                            .to_broadcast((64, 1024)),
                        )
                        for qq in range(2):
                            qs0 = qh * 1024 + qq * 512
                            nc.vector.tensor_mul(
                                attn_sb[h][:, qs0:qs0 + 512],
                                attn_sb[h][:, qs0:qs0 + 512],
                                rb[:, qq * 512:(qq + 1) * 512],
                            )

        # ---- phase 3: output projection (row-parallel partial) ----
        with ExitStack() as c3:
            stage_pool = c3.enter_context(tc.tile_pool(name="ostage", bufs=6))
            ps_o = c3.enter_context(
                tc.tile_pool(name="ps_o", bufs=4, space="PSUM")
            )
            for sb in range(SB):
                for nb in range(2):
                    ps = ps_o.tile([128, 512], F32)
                    for h in range(HG):
                        nc.tensor.matmul(
                            ps[:],
                            attn_sb[h][:, sb * 128:(sb + 1) * 128],
                            wo_sb[h][:, nb * 512:(nb + 1) * 512],
                            start=(h == 0),
                            stop=(h == HG - 1),
                        )
                    st = stage_pool.tile([128, 512], F32, tag="st")
                    if (2 * sb + nb) % 2 == 0:
                        nc.vector.tensor_copy(st[:], ps[:])
                    else:
                        nc.scalar.copy(st[:], ps[:])
                    eng = nc.sync if nb == 0 else nc.gpsimd
                    eng.dma_start(
                        out[sb * 128:(sb + 1) * 128, nb * 512:(nb + 1) * 512],
                        st[:],
                    )
        ctx.close()
    return nc


_NC_CACHE = None


def _get_nc():
    global _NC_CACHE
    if _NC_CACHE is None:
        _NC_CACHE = build_nc()
    return _NC_CACHE


_EXEC_CACHE = None


def _get_executor():
    """Build + jit the SPMD executable once; reuse across kernel() calls.

    Mirrors concourse.bass2jax.run_bass_via_pjrt, which re-jits on every
    call (full retrace + executable reload); caching shaves seconds/call."""
    global _EXEC_CACHE
    if _EXEC_CACHE is not None:
        return _EXEC_CACHE
    import jax
    from jax.sharding import Mesh, PartitionSpec
    from jax.experimental.shard_map import shard_map
    from concourse import bass2jax as b2j

    nc = _get_nc()
    b2j.install_neuronx_cc_hook()
    assert nc.dbg_addr is None
    partition_name = (
        nc.partition_id_tensor.name if nc.partition_id_tensor is not None else None
    )

    in_names, out_names, out_avals = [], [], []
    for alloc in nc.m.functions[0].allocations:
        if not isinstance(alloc, mybir.MemoryLocationSet):
            continue
        name = alloc.memorylocations[0].name
        if alloc.kind == "ExternalInput":
            if name != partition_name:
                in_names.append(name)
        elif alloc.kind == "ExternalOutput":
            out_names.append(name)
            out_avals.append(
                jax.core.ShapedArray(
                    tuple(alloc.tensor_shape), mybir.dt.np(alloc.dtype)
                )
            )
    n_params = len(in_names)
    n_outs = len(out_avals)
    all_names = in_names + out_names
    if partition_name is not None:
        all_names = all_names + [partition_name]

    def _body(*args):
        operands = list(args)
        if partition_name is not None:
            operands.append(b2j.partition_id_tensor())
        outs = b2j._bass_exec_p.bind(
            *operands,
            out_avals=tuple(out_avals),
            in_names=tuple(all_names),
            out_names=tuple(out_names),
            lowering_input_output_aliases=(),
            sim_require_finite=True,
            sim_require_nnan=True,
            nc=nc,
        )
        return tuple(outs)

    devices = jax.devices()[:NCORES]
    mesh = Mesh(np.asarray(devices), ("core",))
    donate = tuple(range(n_params, n_params + n_outs))
    sharded = jax.jit(
        shard_map(
            _body,
            mesh=mesh,
            in_specs=(PartitionSpec("core"),) * (n_params + n_outs),
            out_specs=(PartitionSpec("core"),) * n_outs,
            check_rep=False,
        ),
        donate_argnums=donate,
        keep_unused=True,
    )
    import jax.numpy as jnp

    zero_shardings = [
        jax.sharding.NamedSharding(mesh, PartitionSpec("core"))
    ] * n_outs

    @jax.jit
    def _make_zeros():
        return tuple(
            jax.lax.with_sharding_constraint(
                jnp.zeros((NCORES * a.shape[0], *a.shape[1:]), a.dtype), sh
            )
            for a, sh in zip(out_avals, zero_shardings)
        )

    _EXEC_CACHE = {
        "sharded": sharded,
        "make_zeros": _make_zeros,
        "in_names": in_names,
        "out_names": out_names,
        "out_avals": out_avals,
    }
    return _EXEC_CACHE


def _run_spmd(in_maps):
    ex = _get_executor()
    concat_in = [
        np.concatenate([np.asarray(m[name]) for m in in_maps], axis=0)
        for name in ex["in_names"]
    ]
    concat_zeros = ex["make_zeros"]()
    out_arrs = ex["sharded"](*concat_in, *concat_zeros)
    results = []
    for c in range(NCORES):
        results.append({
            name: np.asarray(out_arrs[i]).reshape(
                NCORES, *ex["out_avals"][i].shape
            )[c]
            for i, name in enumerate(ex["out_names"])
        })
    return results


def _shard_inputs(x, Wq, Wk, Wv, Wo):
    scale = np.float32(1.0 / np.sqrt(HD))
    global _ONES
    _ONES = np.ones((128, SB * HG), dtype=np.float32)
    in_maps = []
    xT_b = [np.ascontiguousarray(x[b].T) for b in range(B)]
    for c in range(NCORES):
        b, g = divmod(c, GROUPS)
        sl = slice(g * E, (g + 1) * E)
        in_maps.append({
            "ones_d": _ONES,
            "xT": xT_b[b],
            "wqT": np.ascontiguousarray(Wq[sl, :].T * scale),
            "wkT": np.ascontiguousarray(Wk[sl, :].T),
            "wvT": np.ascontiguousarray(Wv[sl, :].T),
            "woT": np.ascontiguousarray(Wo[:, sl].T),
        })
    return in_maps


def kernel(x, Wq, Wk, Wv, Wo, bo):
    x = np.asarray(x, dtype=np.float32)
    in_maps = _shard_inputs(
        x,
        np.asarray(Wq, dtype=np.float32),
        np.asarray(Wk, dtype=np.float32),
        np.asarray(Wv, dtype=np.float32),
        np.asarray(Wo, dtype=np.float32),
    )
    results = _run_spmd(in_maps)
    bo = np.asarray(bo, dtype=np.float32)
    out = np.empty((B, S, D), dtype=np.float32)
    for b in range(B):
        acc = np.zeros((S, D), dtype=np.float64)
        for g in range(GROUPS):
            acc += results[b * GROUPS + g]["out_partial"]
        out[b] = (acc + bo.astype(np.float64)).astype(np.float32)
    return out


# revision 25
# speedup vs baseline: 27464.7247x; 22352.2446x over previous
"""Multi-head attention (B=2, S=2048, D=1024, H=16) on 8 Trainium2 NeuronCores.

Sharding: core c handles batch b = c//4 and head group g = c%4 (4 heads, 256
model dims).  Each core computes q/k/v projections for its heads, attention,
and a partial output projection (row-parallel over its 256 head dims); the
host sums the 4 partials per batch and adds the bias.

All activations live in transposed layouts so the tensor engine contraction
dim always sits on SBUF partitions:
  xT  [d, s]  (host pre-transposed)
  qT/kT [e, s] per head-pair tile (heads at partitions 0-63 / 64-127)
  v   [s, e+1] per (s-block, head) with a trailing ones column so the PV
      matmul's 65th output row is the softmax denominator.
Scores are computed transposed [ks, qs]; softmax needs no max-subtraction
(scores ~ N(0,1)) and no free-dim reduction: exp runs elementwise on ACT and
the denominator rides the PV matmul.  Normalization broadcasts 1/denominator
via a DRAM round-trip (partition-broadcast DMA) and one DVE multiply.

Matmuls use float32r (~1.5e-4 rel err, full PE rate for free dims >= 256).
"""

import os
import sys

import numpy as np

for _p in ("/opt/trn_rl_repo", "/root/.axon_site/_ro/trn_rl_repo"):
    if os.path.isdir(_p) and _p not in sys.path:
        sys.path.insert(0, _p)

import bass_rust
import concourse.bass as bass
import concourse.mybir as mybir
import concourse.tile as tile
from concourse.bass_utils import run_bass_kernel_spmd
from concourse.vector_clock import ScopedClock, VectorClock
from contextlib import ExitStack

F32 = mybir.dt.float32
F32R = mybir.dt.float32r
EXP = mybir.ActivationFunctionType.Exp

B = 2
S = 2048
D = 1024
H = 16
HD = 64
NCORES = 8
GROUPS = 4          # head groups (cores per batch)
HG = H // GROUPS    # heads per core = 4
E = HG * HD         # head dims per core = 256
KT = D // 128       # contraction tiles over model dim = 8
SB = S // 128       # s blocks = 16
QB = S // 512       # 512-wide qs blocks = 4

_carrier_counter = [0]


def _split_multi_waits(ordered):
    """This walrus build allows one sync wait per instruction; Tile's wait
    assignment can attach several.  Hoist extras onto same-engine InstNoOp
    carriers placed immediately before the instruction."""
    for bb_name, insts in ordered.items():
        new_list = []
        for inst in insts:
            si = inst.sync_info
            waits = list(si.on_wait) if si is not None else []
            if len(waits) > 1:
                for w in waits[:-1]:
                    _carrier_counter[0] += 1
                    carrier = mybir.InstNoOp(
                        name=f"I-waitc-{_carrier_counter[0]}", ins=[], outs=[]
                    )
                    carrier.engine = inst.engine
                    carrier.sync_info = bass_rust.SyncInfo(on_wait=[w], on_update=[])
                    new_list.append(carrier)
                inst.sync_info = bass_rust.SyncInfo(
                    on_wait=[waits[-1]],
                    on_update=list(si.on_update) if si is not None else [],
                )
            new_list.append(inst)
        ordered[bb_name] = new_list


class _TileContext(tile.TileContext):
    """TileContext adapted to the one-sync-wait-per-instruction walrus."""

    def _lower_ordered_insts(self, ordered):
        _split_multi_waits(ordered)
        return super()._lower_ordered_insts(ordered)

    def _drain_and_barrier(self, tick_clock, wait_clock):
        gc = tick_clock.global_clock
        for proc in range(len(gc)):
            if gc[proc] <= 0:
                continue
            cur = VectorClock([0 if i == proc else gc[i] for i in range(len(gc))])
            nop = self.nc.sync.nop()
            wait_clock.add_sem_waits(
                nop.ins, ScopedClock({None: gc}), ScopedClock({None: cur})
            )
        drain_inst = self.nc.sync.drain()
        wait_clock.add_sem_waits(
            drain_inst.ins, ScopedClock({None: gc}), ScopedClock({None: gc.copy()})
        )
        self.nc.all_engine_barrier()
        assert self.sems is not None
        popped = self.nc._tile_sem_poison_stack.pop()
        assert popped is self._sem_poison
        self.nc.clear_and_free_semaphores(list(self.sems.allocated().values()))
        self.nc.all_engine_barrier()


def build_nc(reps=1):
    nc = bass.Bass()
    xT = nc.declare_dram_parameter("xT", [D, S], F32R, isOutput=False)
    wqT = nc.declare_dram_parameter("wqT", [D, E], F32R, isOutput=False)
    wkT = nc.declare_dram_parameter("wkT", [D, E], F32R, isOutput=False)
    wvT = nc.declare_dram_parameter("wvT", [D, E], F32R, isOutput=False)
    woT = nc.declare_dram_parameter("woT", [E, D], F32R, isOutput=False)
    out = nc.declare_dram_parameter("out_partial", [S, D], F32, isOutput=True)
    ones_d = nc.declare_dram_parameter("ones_d", [128, SB * HG], F32R, isOutput=False)
    den_d = nc.dram_tensor("den_scratch", [HG, S], F32)

    with _TileContext(nc) as tc, ExitStack() as outer:
      for _rep in range(reps):
        ctx = outer.enter_context(ExitStack())
        # ---- persistent activation tiles (live across phases) ----
        act_pool = ctx.enter_context(tc.tile_pool(name="acts", bufs=1))
        qT_sb = [act_pool.tile([128, S], F32R, tag=f"qT{m}", name=f"qT{m}") for m in range(2)]
        kT_sb = [act_pool.tile([128, S], F32R, tag=f"kT{m}", name=f"kT{m}") for m in range(2)]
        v_sb = act_pool.tile([128, SB, HG, HD + 1], F32R, tag="v")
        wo_sb = [act_pool.tile([64, D], F32R, tag=f"wo{h}", name=f"wo{h}") for h in range(HG)]

        # ---- phase 1: load inputs, project q/k/v ----
        with ExitStack() as c1:
            in_pool = c1.enter_context(tc.tile_pool(name="ins", bufs=1))
            ps_qk = c1.enter_context(tc.tile_pool(name="ps_qk", bufs=2, space="PSUM"))
            ps_v = c1.enter_context(tc.tile_pool(name="ps_v", bufs=2, space="PSUM"))

            x_sb = in_pool.tile([128, KT, S], F32R, tag="x")
            wq_sb = in_pool.tile([128, KT, E], F32R, tag="wq")
            wk_sb = in_pool.tile([128, KT, E], F32R, tag="wk")
            wv_sb = in_pool.tile([128, KT, E], F32R, tag="wv")
            # split loads across the two HWDGE queues (SP + ACT) so the
            # phase-1 matmul ramp isn't paced by one DMA queue
            for k in range(KT):
                eng = nc.sync if k % 2 == 0 else nc.gpsimd
                eng2 = nc.gpsimd if k % 2 == 0 else nc.sync
                eng.dma_start(x_sb[:, k, :], xT[k * 128:(k + 1) * 128, :])
                eng2.dma_start(wq_sb[:, k, :], wqT[k * 128:(k + 1) * 128, :])
                eng.dma_start(wk_sb[:, k, :], wkT[k * 128:(k + 1) * 128, :])
                eng2.dma_start(wv_sb[:, k, :], wvT[k * 128:(k + 1) * 128, :])
            for h in range(HG):
                nc.gpsimd.dma_start(wo_sb[h][:, :], woT[h * 64:(h + 1) * 64, :])

            # ones column for the softmax-denominator rows of v
            nc.sync.dma_start(
                v_sb[:, :, :, HD],
                ones_d[:, :].rearrange("p (s h) -> p s h", s=SB),
            )

            # qT / kT: W-stationary, out [e(128), s]
            for w_sb, dst in ((wq_sb, qT_sb), (wk_sb, kT_sb)):
                for m in range(2):
                    for nb in range(QB):
                        ps = ps_qk.tile([128, 512], F32)
                        for k in range(KT):
                            nc.tensor.matmul(
                                ps[:],
                                w_sb[:, k, m * 128:(m + 1) * 128],
                                x_sb[:, k, nb * 512:(nb + 1) * 512],
                                start=(k == 0),
                                stop=(k == KT - 1),
                            )
                        # alternate copies between DVE and ACT (ACT idle here)
                        if nb % 2 == 0:
                            nc.vector.tensor_copy(
                                dst[m][:, nb * 512:(nb + 1) * 512], ps[:]
                            )
                        else:
                            nc.scalar.copy(
                                dst[m][:, nb * 512:(nb + 1) * 512], ps[:]
                            )
            # v: x-stationary, out [s(128), e]
            for sb in range(SB):
                ps = ps_v.tile([128, E], F32)
                for k in range(KT):
                    nc.tensor.matmul(
                        ps[:],
                        x_sb[:, k, sb * 128:(sb + 1) * 128],
                        wv_sb[:, k, :],
                        start=(k == 0),
                        stop=(k == KT - 1),
                    )
                if sb % 2 == 0:
                    nc.vector.tensor_copy(
                        v_sb[:, sb, :, 0:HD],
                        ps[:].rearrange("p (h e) -> p h e", h=HG),
                    )
                else:
                    nc.scalar.copy(
                        v_sb[:, sb, :, 0:HD],
                        ps[:].rearrange("p (h e) -> p h e", h=HG),
                    )

        # ---- phase 2: attention per head-pair (m), per qs-half ----
        attn_pool = ctx.enter_context(tc.tile_pool(name="attn", bufs=1))
        attn_sb = [attn_pool.tile([64, S], F32R, tag=f"at{h}", name=f"at{h}") for h in range(HG)]
        with ExitStack() as c2:
            p_pool = c2.enter_context(tc.tile_pool(name="pexp", bufs=4))
            rb_pool = c2.enter_context(tc.tile_pool(name="rbc", bufs=2))
            den_pool = c2.enter_context(tc.tile_pool(name="den", bufs=2))
            ps_pv = c2.enter_context(tc.tile_pool(name="ps_pv", bufs=1, space="PSUM"))
            ps_sc = c2.enter_context(tc.tile_pool(name="ps_sc", bufs=1, space="PSUM"))

            for m in range(2):
                for qh in range(2):  # qs halves of 1024
                    out_ps = [
                        [ps_pv.tile([128, 512], F32, tag=f"pv{r}{qq}", name=f"pv{r}{qq}") for qq in range(2)]
                        for r in range(2)
                    ]
                    for ksb in range(SB):
                        sc = [ps_sc.tile([128, 1024], F32, tag=f"sc{r}", name=f"sc{r}") for r in range(2)]
                        for qq in range(2):
                            for r in range(2):
                                nc.tensor.matmul(
                                    sc[r][:, qq * 512:(qq + 1) * 512],
                                    kT_sb[m][64 * r:64 * r + 64,
                                             ksb * 128:(ksb + 1) * 128],
                                    qT_sb[m][64 * r:64 * r + 64,
                                             qh * 1024 + qq * 512:
                                             qh * 1024 + (qq + 1) * 512],
                                    start=True,
                                    stop=True,
                                )
                        pt = [None, None]
                        for r in range(2):
                            pt[r] = p_pool.tile([128, 1024], F32R, tag="p", name=f"p{r}")
                            nc.scalar.activation(pt[r][:], sc[r][:], EXP)
                        for r in range(2):
                            for qq in range(2):
                                nc.tensor.matmul(
                                    out_ps[r][qq][0:HD + 1, :],
                                    v_sb[:, ksb, 2 * m + r, :],
                                    pt[r][:, qq * 512:(qq + 1) * 512],
                                    start=(ksb == 0),
                                    stop=(ksb == SB - 1),
                                )
                    # normalize: attn = attnU * (1/denom), denom = row 64.
                    # Copy attnU + denom out of PSUM immediately (frees the
                    # accumulation banks for the next group); the broadcasted
                    # reciprocal multiply happens later in SBUF, off the
                    # PSUM critical path.
                    for r in range(2):
                        h = 2 * m + r
                        den = den_pool.tile([128, 1024], F32, tag="den")
                        for qq in range(2):
                            qs0 = qh * 1024 + qq * 512
                            nc.vector.tensor_copy(
                                den[64:65, qq * 512:(qq + 1) * 512],
                                out_ps[r][qq][64:65, :],
                            )
                            nc.vector.tensor_copy(
                                attn_sb[h][:, qs0:qs0 + 512],
                                out_ps[r][qq][0:64, :],
                            )
                        nc.vector.reciprocal(den[64:65, :], den[64:65, :])
                        nc.sync.dma_start(
                            den_d[h:h + 1, qh * 1024:(qh + 1) * 1024], den[64:65, :]
                        )
                        rb = rb_pool.tile([64, 1024], F32, tag="rb")
                        nc.sync.dma_start(
                            rb[:, :],
                            den_d[h:h + 1, qh * 1024:(qh + 1) * 1024]
                            .to_broadcast((64, 1024)),
                        )
                        for qq in range(2):
                            qs0 = qh * 1024 + qq * 512
                            nc.vector.tensor_mul(
                                attn_sb[h][:, qs0:qs0 + 512],
                                attn_sb[h][:, qs0:qs0 + 512],
                                rb[:, qq * 512:(qq + 1) * 512],
                            )

        # ---- phase 3: output projection (row-parallel partial) ----
        with ExitStack() as c3:
            stage_pool = c3.enter_context(tc.tile_pool(name="ostage", bufs=6))
            ps_o = c3.enter_context(
                tc.tile_pool(name="ps_o", bufs=4, space="PSUM")
            )
            for sb in range(SB):
                for nb in range(2):
                    ps = ps_o.tile([128, 512], F32)
                    for h in range(HG):
                        nc.tensor.matmul(
                            ps[:],
                            attn_sb[h][:, sb * 128:(sb + 1) * 128],
                            wo_sb[h][:, nb * 512:(nb + 1) * 512],
                            start=(h == 0),
                            stop=(h == HG - 1),
                        )
                    st = stage_pool.tile([128, 512], F32, tag="st")
                    if (2 * sb + nb) % 2 == 0:
                        nc.vector.tensor_copy(st[:], ps[:])
                    else:
                        nc.scalar.copy(st[:], ps[:])
                    eng = nc.sync if nb == 0 else nc.gpsimd
                    eng.dma_start(
                        out[sb * 128:(sb + 1) * 128, nb * 512:(nb + 1) * 512],
                        st[:],
                    )
        ctx.close()
    return nc


_NC_CACHE = None


def _get_nc():
    global _NC_CACHE
    if _NC_CACHE is None:
        _NC_CACHE = build_nc()
    return _NC_CACHE


_EXEC_CACHE = None


def _get_executor():
    """Build + jit the SPMD executable once; reuse across kernel() calls.

    Mirrors concourse.bass2jax.run_bass_via_pjrt, which re-jits on every
    call (full retrace + executable reload); caching shaves seconds/call."""
    global _EXEC_CACHE
    if _EXEC_CACHE is not None:
        return _EXEC_CACHE
    import jax
    from jax.sharding import Mesh, PartitionSpec
    from jax.experimental.shard_map import shard_map
    from concourse import bass2jax as b2j

    nc = _get_nc()
    b2j.install_neuronx_cc_hook()
    assert nc.dbg_addr is None
    partition_name = (
        nc.partition_id_tensor.name if nc.partition_id_tensor is not None else None
    )

    in_names, out_names, out_avals = [], [], []
    for alloc in nc.m.functions[0].allocations:
        if not isinstance(alloc, mybir.MemoryLocationSet):
            continue
        name = alloc.memorylocations[0].name
        if alloc.kind == "ExternalInput":
            if name != partition_name:
                in_names.append(name)
        elif alloc.kind == "ExternalOutput":
            out_names.append(name)
            out_avals.append(
                jax.core.ShapedArray(
                    tuple(alloc.tensor_shape), mybir.dt.np(alloc.dtype)
                )
            )
    n_params = len(in_names)
    n_outs = len(out_avals)
    all_names = in_names + out_names
    if partition_name is not None:
        all_names = all_names + [partition_name]

    def _body(*args):
        operands = list(args)
        if partition_name is not None:
            operands.append(b2j.partition_id_tensor())
        outs = b2j._bass_exec_p.bind(
            *operands,
            out_avals=tuple(out_avals),
            in_names=tuple(all_names),
            out_names=tuple(out_names),
            lowering_input_output_aliases=(),
            sim_require_finite=True,
            sim_require_nnan=True,
            nc=nc,
        )
        return tuple(outs)

    devices = jax.devices()[:NCORES]
    mesh = Mesh(np.asarray(devices), ("core",))
    donate = tuple(range(n_params, n_params + n_outs))
    sharded = jax.jit(
        shard_map(
            _body,
            mesh=mesh,
            in_specs=(PartitionSpec("core"),) * (n_params + n_outs),
            out_specs=(PartitionSpec("core"),) * n_outs,
            check_rep=False,
        ),
        donate_argnums=donate,
        keep_unused=True,
    )
    import jax.numpy as jnp

    zero_shardings = [
        jax.sharding.NamedSharding(mesh, PartitionSpec("core"))
    ] * n_outs

    @jax.jit
    def _make_zeros():
        return tuple(
            jax.lax.with_sharding_constraint(
                jnp.zeros((NCORES * a.shape[0], *a.shape[1:]), a.dtype), sh
            )
            for a, sh in zip(out_avals, zero_shardings)
        )

    _EXEC_CACHE = {
        "sharded": sharded,
        "make_zeros": _make_zeros,
        "in_names": in_names,
        "out_names": out_names,
        "out_avals": out_avals,
    }
    return _EXEC_CACHE


def _run_spmd(in_maps):
    ex = _get_executor()
    concat_in = [
        np.concatenate([np.asarray(m[name]) for m in in_maps], axis=0)
        for name in ex["in_names"]
    ]
    concat_zeros = ex["make_zeros"]()
    out_arrs = ex["sharded"](*concat_in, *concat_zeros)
    results = []
    for c in range(NCORES):
        results.append({
            name: np.asarray(out_arrs[i]).reshape(
                NCORES, *ex["out_avals"][i].shape
            )[c]
            for i, name in enumerate(ex["out_names"])
        })
    return results


def _shard_inputs(x, Wq, Wk, Wv, Wo):
    scale = np.float32(1.0 / np.sqrt(HD))
    global _ONES
    _ONES = np.ones((128, SB * HG), dtype=np.float32)
    in_maps = []
    xT_b = [np.ascontiguousarray(x[b].T) for b in range(B)]
    for c in range(NCORES):
        b, g = divmod(c, GROUPS)
        sl = slice(g * E, (g + 1) * E)
        in_maps.append({
            "ones_d": _ONES,
            "xT": xT_b[b],
            "wqT": np.ascontiguousarray(Wq[sl, :].T * scale),
            "wkT": np.ascontiguousarray(Wk[sl, :].T),
            "wvT": np.ascontiguousarray(Wv[sl, :].T),
            "woT": np.ascontiguousarray(Wo[:, sl].T),
        })
    return in_maps


_FAST_PATH_OK = True


def kernel(x, Wq, Wk, Wv, Wo, bo):
    global _FAST_PATH_OK
    x = np.asarray(x, dtype=np.float32)
    in_maps = _shard_inputs(
        x,
        np.asarray(Wq, dtype=np.float32),
        np.asarray(Wk, dtype=np.float32),
        np.asarray(Wv, dtype=np.float32),
        np.asarray(Wo, dtype=np.float32),
    )
    results = None
    if _FAST_PATH_OK:
        try:
            results = _run_spmd(in_maps)
        except Exception:
            _FAST_PATH_OK = False
    if results is None:
        # portable fallback: stock SPMD runner (handles native-device
        # environments and anything the cached-PJRT fast path can't)
        results = run_bass_kernel_spmd(
            _get_nc(), in_maps, list(range(NCORES))
        ).results
    bo = np.asarray(bo, dtype=np.float32)
    out = np.empty((B, S, D), dtype=np.float32)
    for b in range(B):
        acc = np.zeros((S, D), dtype=np.float64)
        for g in range(GROUPS):
            acc += results[b * GROUPS + g]["out_partial"]
        out[b] = (acc + bo.astype(np.float64)).astype(np.float32)
    return out


# revision 27
# speedup vs baseline: 28138.1568x; 1.0245x over previous
"""Multi-head attention (B=2, S=2048, D=1024, H=16) on 8 Trainium2 NeuronCores.

Sharding: core c handles batch b = c//4 and head group g = c%4 (4 heads, 256
model dims).  Each core computes q/k/v projections for its heads, attention,
and a partial output projection (row-parallel over its 256 head dims); the
host sums the 4 partials per batch and adds the bias.

All activations live in transposed layouts so the tensor engine contraction
dim always sits on SBUF partitions:
  xT  [d, s]  (host pre-transposed)
  qT/kT [e, s] per head-pair tile (heads at partitions 0-63 / 64-127)
  v   [s, e+1] per (s-block, head) with a trailing ones column so the PV
      matmul's 65th output row is the softmax denominator.
Scores are computed transposed [ks, qs]; softmax needs no max-subtraction
(scores ~ N(0,1)) and no free-dim reduction: exp runs elementwise on ACT and
the denominator rides the PV matmul.  Normalization broadcasts 1/denominator
via a DRAM round-trip (partition-broadcast DMA) and one DVE multiply.

Matmuls use float32r (~1.5e-4 rel err, full PE rate for free dims >= 256).
"""

import os
import sys

import numpy as np

for _p in ("/opt/trn_rl_repo", "/root/.axon_site/_ro/trn_rl_repo"):
    if os.path.isdir(_p) and _p not in sys.path:
        sys.path.insert(0, _p)

import bass_rust
import concourse.bass as bass
import concourse.mybir as mybir
import concourse.tile as tile
from concourse.bass_utils import run_bass_kernel_spmd
from concourse.vector_clock import ScopedClock, VectorClock
from contextlib import ExitStack

F32 = mybir.dt.float32
F32R = mybir.dt.float32r
EXP = mybir.ActivationFunctionType.Exp

B = 2
S = 2048
D = 1024
H = 16
HD = 64
NCORES = 8
GROUPS = 4          # head groups (cores per batch)
HG = H // GROUPS    # heads per core = 4
E = HG * HD         # head dims per core = 256
KT = D // 128       # contraction tiles over model dim = 8
SB = S // 128       # s blocks = 16
QB = S // 512       # 512-wide qs blocks = 4

_carrier_counter = [0]


def _split_multi_waits(ordered):
    """This walrus build allows one sync wait per instruction; Tile's wait
    assignment can attach several.  Hoist extras onto same-engine InstNoOp
    carriers placed immediately before the instruction."""
    for bb_name, insts in ordered.items():
        new_list = []
        for inst in insts:
            si = inst.sync_info
            waits = list(si.on_wait) if si is not None else []
            if len(waits) > 1:
                for w in waits[:-1]:
                    _carrier_counter[0] += 1
                    carrier = mybir.InstNoOp(
                        name=f"I-waitc-{_carrier_counter[0]}", ins=[], outs=[]
                    )
                    carrier.engine = inst.engine
                    carrier.sync_info = bass_rust.SyncInfo(on_wait=[w], on_update=[])
                    new_list.append(carrier)
                inst.sync_info = bass_rust.SyncInfo(
                    on_wait=[waits[-1]],
                    on_update=list(si.on_update) if si is not None else [],
                )
            new_list.append(inst)
        ordered[bb_name] = new_list


class _TileContext(tile.TileContext):
    """TileContext adapted to the one-sync-wait-per-instruction walrus."""

    def _lower_ordered_insts(self, ordered):
        _split_multi_waits(ordered)
        return super()._lower_ordered_insts(ordered)

    def _drain_and_barrier(self, tick_clock, wait_clock):
        gc = tick_clock.global_clock
        for proc in range(len(gc)):
            if gc[proc] <= 0:
                continue
            cur = VectorClock([0 if i == proc else gc[i] for i in range(len(gc))])
            nop = self.nc.sync.nop()
            wait_clock.add_sem_waits(
                nop.ins, ScopedClock({None: gc}), ScopedClock({None: cur})
            )
        drain_inst = self.nc.sync.drain()
        wait_clock.add_sem_waits(
            drain_inst.ins, ScopedClock({None: gc}), ScopedClock({None: gc.copy()})
        )
        self.nc.all_engine_barrier()
        assert self.sems is not None
        popped = self.nc._tile_sem_poison_stack.pop()
        assert popped is self._sem_poison
        self.nc.clear_and_free_semaphores(list(self.sems.allocated().values()))
        self.nc.all_engine_barrier()


def build_nc(reps=1):
    nc = bass.Bass()
    xT = nc.declare_dram_parameter("xT", [D, S], F32R, isOutput=False)
    wqT = nc.declare_dram_parameter("wqT", [D, E], F32R, isOutput=False)
    wkT = nc.declare_dram_parameter("wkT", [D, E], F32R, isOutput=False)
    wvT = nc.declare_dram_parameter("wvT", [D, E], F32R, isOutput=False)
    woT = nc.declare_dram_parameter("woT", [E, D], F32R, isOutput=False)
    out = nc.declare_dram_parameter("out_partial", [S, D], F32, isOutput=True)
    ones_d = nc.declare_dram_parameter("ones_d", [128, SB * HG], F32R, isOutput=False)
    den_d = nc.dram_tensor("den_scratch", [HG, S], F32)

    with _TileContext(nc) as tc, ExitStack() as outer:
      for _rep in range(reps):
        ctx = outer.enter_context(ExitStack())
        # ---- persistent activation tiles (live across phases) ----
        act_pool = ctx.enter_context(tc.tile_pool(name="acts", bufs=1))
        qT_sb = [act_pool.tile([128, S], F32R, tag=f"qT{m}", name=f"qT{m}") for m in range(2)]
        kT_sb = [act_pool.tile([128, S], F32R, tag=f"kT{m}", name=f"kT{m}") for m in range(2)]
        v_sb = act_pool.tile([128, SB, HG, HD + 1], F32R, tag="v")
        wo_sb = [act_pool.tile([64, D], F32R, tag=f"wo{h}", name=f"wo{h}") for h in range(HG)]

        # ---- phase 1: load inputs, project q/k/v ----
        with ExitStack() as c1:
            in_pool = c1.enter_context(tc.tile_pool(name="ins", bufs=1))
            ps_qk = c1.enter_context(tc.tile_pool(name="ps_qk", bufs=2, space="PSUM"))
            ps_v = c1.enter_context(tc.tile_pool(name="ps_v", bufs=2, space="PSUM"))

            x_sb = in_pool.tile([128, KT, S], F32R, tag="x")
            wq_sb = in_pool.tile([128, KT, E], F32R, tag="wq")
            wk_sb = in_pool.tile([128, KT, E], F32R, tag="wk")
            wv_sb = in_pool.tile([128, KT, E], F32R, tag="wv")
            # weights first (small), then x in 256KB qs-major chunks split
            # across both HWDGE queues — the first qT psum group needs only
            # wq + the first 8 x chunks instead of the whole 8MB of x
            for k in range(KT):
                eng = nc.sync if k % 2 == 0 else nc.gpsimd
                eng2 = nc.gpsimd if k % 2 == 0 else nc.sync
                eng.dma_start(wq_sb[:, k, :], wqT[k * 128:(k + 1) * 128, :])
                eng2.dma_start(wk_sb[:, k, :], wkT[k * 128:(k + 1) * 128, :])
                eng.dma_start(wv_sb[:, k, :], wvT[k * 128:(k + 1) * 128, :])
            for nb in range(QB):
                for k in range(KT):
                    eng = nc.sync if k % 2 == 0 else nc.gpsimd
                    eng.dma_start(
                        x_sb[:, k, nb * 512:(nb + 1) * 512],
                        xT[k * 128:(k + 1) * 128, nb * 512:(nb + 1) * 512],
                    )
            for h in range(HG):
                nc.gpsimd.dma_start(wo_sb[h][:, :], woT[h * 64:(h + 1) * 64, :])

            # ones column for the softmax-denominator rows of v
            nc.sync.dma_start(
                v_sb[:, :, :, HD],
                ones_d[:, :].rearrange("p (s h) -> p s h", s=SB),
            )

            # qT / kT: W-stationary, out [e(128), s]; qs-major so compute
            # starts as soon as the first x column-block lands
            for nb in range(QB):
                for w_sb, dst in ((wq_sb, qT_sb), (wk_sb, kT_sb)):
                    for m in range(2):
                        ps = ps_qk.tile([128, 512], F32)
                        for k in range(KT):
                            nc.tensor.matmul(
                                ps[:],
                                w_sb[:, k, m * 128:(m + 1) * 128],
                                x_sb[:, k, nb * 512:(nb + 1) * 512],
                                start=(k == 0),
                                stop=(k == KT - 1),
                            )
                        # alternate copies between DVE and ACT (ACT idle here)
                        if (nb + m) % 2 == 0:
                            nc.vector.tensor_copy(
                                dst[m][:, nb * 512:(nb + 1) * 512], ps[:]
                            )
                        else:
                            nc.scalar.copy(
                                dst[m][:, nb * 512:(nb + 1) * 512], ps[:]
                            )
            # v: x-stationary, out [s(128), e]
            for sb in range(SB):
                ps = ps_v.tile([128, E], F32)
                for k in range(KT):
                    nc.tensor.matmul(
                        ps[:],
                        x_sb[:, k, sb * 128:(sb + 1) * 128],
                        wv_sb[:, k, :],
                        start=(k == 0),
                        stop=(k == KT - 1),
                    )
                if sb % 2 == 0:
                    nc.vector.tensor_copy(
                        v_sb[:, sb, :, 0:HD],
                        ps[:].rearrange("p (h e) -> p h e", h=HG),
                    )
                else:
                    nc.scalar.copy(
                        v_sb[:, sb, :, 0:HD],
                        ps[:].rearrange("p (h e) -> p h e", h=HG),
                    )

        # ---- phase 2: attention per head-pair (m), per qs-half ----
        # attn lands in paired [128, S] tiles (head 2m rows 0-63, head 2m+1
        # rows 64-127) so the projection contracts K=128 per pair.  Odd heads
        # can't be written to partitions 64-127 by compute engines, so they
        # stage at partitions 0-63 and round-trip through DRAM (DMA shifts
        # partitions freely).
        attn_pool = ctx.enter_context(tc.tile_pool(name="attn", bufs=1))
        attn_pair = [attn_pool.tile([128, S], F32R, tag=f"ap{m}", name=f"ap{m}") for m in range(2)]
        attn_odd = [attn_pool.tile([64, S], F32R, tag=f"ao{m}", name=f"ao{m}") for m in range(2)]
        with ExitStack() as c2:
            p_pool = c2.enter_context(tc.tile_pool(name="pexp", bufs=4))
            rb_pool = c2.enter_context(tc.tile_pool(name="rbc", bufs=2))
            den_pool = c2.enter_context(tc.tile_pool(name="den", bufs=2))
            ps_pv = c2.enter_context(tc.tile_pool(name="ps_pv", bufs=1, space="PSUM"))
            ps_sc = c2.enter_context(tc.tile_pool(name="ps_sc", bufs=1, space="PSUM"))

            for m in range(2):
                for qh in range(2):  # qs halves of 1024
                    out_ps = [
                        [ps_pv.tile([128, 512], F32, tag=f"pv{r}{qq}", name=f"pv{r}{qq}") for qq in range(2)]
                        for r in range(2)
                    ]
                    for ksb in range(SB):
                        sc = [ps_sc.tile([128, 1024], F32, tag=f"sc{r}", name=f"sc{r}") for r in range(2)]
                        for qq in range(2):
                            for r in range(2):
                                nc.tensor.matmul(
                                    sc[r][:, qq * 512:(qq + 1) * 512],
                                    kT_sb[m][64 * r:64 * r + 64,
                                             ksb * 128:(ksb + 1) * 128],
                                    qT_sb[m][64 * r:64 * r + 64,
                                             qh * 1024 + qq * 512:
                                             qh * 1024 + (qq + 1) * 512],
                                    start=True,
                                    stop=True,
                                )
                        pt = [None, None]
                        for r in range(2):
                            pt[r] = p_pool.tile([128, 1024], F32R, tag="p", name=f"p{r}")
                            nc.scalar.activation(pt[r][:], sc[r][:], EXP)
                        for r in range(2):
                            for qq in range(2):
                                nc.tensor.matmul(
                                    out_ps[r][qq][0:HD + 1, :],
                                    v_sb[:, ksb, 2 * m + r, :],
                                    pt[r][:, qq * 512:(qq + 1) * 512],
                                    start=(ksb == 0),
                                    stop=(ksb == SB - 1),
                                )
                    # normalize: attn = attnU * (1/denom), denom = row 64.
                    # Copy attnU + denom out of PSUM immediately (frees the
                    # accumulation banks for the next group); the broadcasted
                    # reciprocal multiply happens later in SBUF, off the
                    # PSUM critical path.
                    for r in range(2):
                        h = 2 * m + r
                        den = den_pool.tile([128, 1024], F32, tag="den")
                        for qq in range(2):
                            qs0 = qh * 1024 + qq * 512
                            nc.vector.tensor_copy(
                                den[64:65, qq * 512:(qq + 1) * 512],
                                out_ps[r][qq][64:65, :],
                            )
                            nc.vector.tensor_copy(
                                attn_sb[h][:, qs0:qs0 + 512],
                                out_ps[r][qq][0:64, :],
                            )
                        nc.vector.reciprocal(den[64:65, :], den[64:65, :])
                        nc.sync.dma_start(
                            den_d[h:h + 1, qh * 1024:(qh + 1) * 1024], den[64:65, :]
                        )
                        rb = rb_pool.tile([64, 1024], F32, tag="rb")
                        nc.sync.dma_start(
                            rb[:, :],
                            den_d[h:h + 1, qh * 1024:(qh + 1) * 1024]
                            .to_broadcast((64, 1024)),
                        )
                        for qq in range(2):
                            qs0 = qh * 1024 + qq * 512
                            nc.vector.tensor_mul(
                                attn_sb[h][:, qs0:qs0 + 512],
                                attn_sb[h][:, qs0:qs0 + 512],
                                rb[:, qq * 512:(qq + 1) * 512],
                            )

        # ---- phase 3: output projection (row-parallel partial) ----
        with ExitStack() as c3:
            stage_pool = c3.enter_context(tc.tile_pool(name="ostage", bufs=6))
            ps_o = c3.enter_context(
                tc.tile_pool(name="ps_o", bufs=4, space="PSUM")
            )
            for sb in range(SB):
                for nb in range(2):
                    ps = ps_o.tile([128, 512], F32)
                    for h in range(HG):
                        nc.tensor.matmul(
                            ps[:],
                            attn_sb[h][:, sb * 128:(sb + 1) * 128],
                            wo_sb[h][:, nb * 512:(nb + 1) * 512],
                            start=(h == 0),
                            stop=(h == HG - 1),
                        )
                    st = stage_pool.tile([128, 512], F32, tag="st")
                    if (2 * sb + nb) % 2 == 0:
                        nc.vector.tensor_copy(st[:], ps[:])
                    else:
                        nc.scalar.copy(st[:], ps[:])
                    eng = nc.sync if nb == 0 else nc.gpsimd
                    eng.dma_start(
                        out[sb * 128:(sb + 1) * 128, nb * 512:(nb + 1) * 512],
                        st[:],
                    )
        ctx.close()
    return nc


_NC_CACHE = None


def _get_nc():
    global _NC_CACHE
    if _NC_CACHE is None:
        _NC_CACHE = build_nc()
    return _NC_CACHE


_EXEC_CACHE = None


def _get_executor():
    """Build + jit the SPMD executable once; reuse across kernel() calls.

    Mirrors concourse.bass2jax.run_bass_via_pjrt, which re-jits on every
    call (full retrace + executable reload); caching shaves seconds/call."""
    global _EXEC_CACHE
    if _EXEC_CACHE is not None:
        return _EXEC_CACHE
    import jax
    from jax.sharding import Mesh, PartitionSpec
    from jax.experimental.shard_map import shard_map
    from concourse import bass2jax as b2j

    nc = _get_nc()
    b2j.install_neuronx_cc_hook()
    assert nc.dbg_addr is None
    partition_name = (
        nc.partition_id_tensor.name if nc.partition_id_tensor is not None else None
    )

    in_names, out_names, out_avals = [], [], []
    for alloc in nc.m.functions[0].allocations:
        if not isinstance(alloc, mybir.MemoryLocationSet):
            continue
        name = alloc.memorylocations[0].name
        if alloc.kind == "ExternalInput":
            if name != partition_name:
                in_names.append(name)
        elif alloc.kind == "ExternalOutput":
            out_names.append(name)
            out_avals.append(
                jax.core.ShapedArray(
                    tuple(alloc.tensor_shape), mybir.dt.np(alloc.dtype)
                )
            )
    n_params = len(in_names)
    n_outs = len(out_avals)
    all_names = in_names + out_names
    if partition_name is not None:
        all_names = all_names + [partition_name]

    def _body(*args):
        operands = list(args)
        if partition_name is not None:
            operands.append(b2j.partition_id_tensor())
        outs = b2j._bass_exec_p.bind(
            *operands,
            out_avals=tuple(out_avals),
            in_names=tuple(all_names),
            out_names=tuple(out_names),
            lowering_input_output_aliases=(),
            sim_require_finite=True,
            sim_require_nnan=True,
            nc=nc,
        )
        return tuple(outs)

    devices = jax.devices()[:NCORES]
    mesh = Mesh(np.asarray(devices), ("core",))
    donate = tuple(range(n_params, n_params + n_outs))
    sharded = jax.jit(
        shard_map(
            _body,
            mesh=mesh,
            in_specs=(PartitionSpec("core"),) * (n_params + n_outs),
            out_specs=(PartitionSpec("core"),) * n_outs,
            check_rep=False,
        ),
        donate_argnums=donate,
        keep_unused=True,
    )
    import jax.numpy as jnp

    zero_shardings = [
        jax.sharding.NamedSharding(mesh, PartitionSpec("core"))
    ] * n_outs

    @jax.jit
    def _make_zeros():
        return tuple(
            jax.lax.with_sharding_constraint(
                jnp.zeros((NCORES * a.shape[0], *a.shape[1:]), a.dtype), sh
            )
            for a, sh in zip(out_avals, zero_shardings)
        )

    _EXEC_CACHE = {
        "sharded": sharded,
        "make_zeros": _make_zeros,
        "in_names": in_names,
        "out_names": out_names,
        "out_avals": out_avals,
    }
    return _EXEC_CACHE


def _run_spmd(in_maps):
    ex = _get_executor()
    concat_in = [
        np.concatenate([np.asarray(m[name]) for m in in_maps], axis=0)
        for name in ex["in_names"]
    ]
    concat_zeros = ex["make_zeros"]()
    out_arrs = ex["sharded"](*concat_in, *concat_zeros)
    results = []
    for c in range(NCORES):
        results.append({
            name: np.asarray(out_arrs[i]).reshape(
                NCORES, *ex["out_avals"][i].shape
            )[c]
            for i, name in enumerate(ex["out_names"])
        })
    return results


def _shard_inputs(x, Wq, Wk, Wv, Wo):
    scale = np.float32(1.0 / np.sqrt(HD))
    global _ONES
    _ONES = np.ones((128, SB * HG), dtype=np.float32)
    in_maps = []
    xT_b = [np.ascontiguousarray(x[b].T) for b in range(B)]
    for c in range(NCORES):
        b, g = divmod(c, GROUPS)
        sl = slice(g * E, (g + 1) * E)
        in_maps.append({
            "ones_d": _ONES,
            "xT": xT_b[b],
            "wqT": np.ascontiguousarray(Wq[sl, :].T * scale),
            "wkT": np.ascontiguousarray(Wk[sl, :].T),
            "wvT": np.ascontiguousarray(Wv[sl, :].T),
            "woT": np.ascontiguousarray(Wo[:, sl].T),
        })
    return in_maps


_FAST_PATH_OK = True


def kernel(x, Wq, Wk, Wv, Wo, bo):
    global _FAST_PATH_OK
    x = np.asarray(x, dtype=np.float32)
    in_maps = _shard_inputs(
        x,
        np.asarray(Wq, dtype=np.float32),
        np.asarray(Wk, dtype=np.float32),
        np.asarray(Wv, dtype=np.float32),
        np.asarray(Wo, dtype=np.float32),
    )
    results = None
    if _FAST_PATH_OK:
        try:
            results = _run_spmd(in_maps)
        except Exception:
            _FAST_PATH_OK = False
    if results is None:
        # portable fallback: stock SPMD runner (handles native-device
        # environments and anything the cached-PJRT fast path can't)
        results = run_bass_kernel_spmd(
            _get_nc(), in_maps, list(range(NCORES))
        ).results
    bo = np.asarray(bo, dtype=np.float32)
    out = np.empty((B, S, D), dtype=np.float32)
    for b in range(B):
        acc = np.zeros((S, D), dtype=np.float64)
        for g in range(GROUPS):
            acc += results[b * GROUPS + g]["out_partial"]
        out[b] = (acc + bo.astype(np.float64)).astype(np.float32)
    return out


# revision 39
# speedup vs baseline: 29464.2588x; 1.0471x over previous
"""Multi-head attention (B=2, S=2048, D=1024, H=16) on 8 Trainium2 NeuronCores.

Sharding: core c handles batch b = c//4 and head group g = c%4 (4 heads, 256
model dims).  Each core computes q/k/v projections for its heads, attention,
and a partial output projection (row-parallel over its 256 head dims); the
host sums the 4 partials per batch and adds the bias.

All activations live in transposed layouts so the tensor engine contraction
dim always sits on SBUF partitions:
  xT  [d, s]  (host pre-transposed)
  qT/kT [e, s] per head-pair tile (heads at partitions 0-63 / 64-127)
  v   [s, e+1] per (s-block, head) with a trailing ones column so the PV
      matmul's 65th output row is the softmax denominator.
Scores are computed transposed [ks, qs]; softmax needs no max-subtraction
(scores ~ N(0,1)) and no free-dim reduction: exp runs elementwise on ACT and
the denominator rides the PV matmul.  Normalization broadcasts 1/denominator
via a DRAM round-trip (partition-broadcast DMA) and one DVE multiply.
Normalized attention is packed into head-pair [128, S] tiles (odd heads
DRAM-round-tripped to partitions 64-127) so the output projection contracts
K=128 per pair.

Matmuls use float32r (~1.5e-4 rel err, full PE rate for free dims >= 256).
Cost-model makespan per core: ~225 us (exp on ACT is the structural floor at
~152 us; PSUM's 8 banks preclude overlapping the projection under phase 2).
"""

import os
import sys

import numpy as np

for _p in ("/opt/trn_rl_repo", "/root/.axon_site/_ro/trn_rl_repo"):
    if os.path.isdir(_p) and _p not in sys.path:
        sys.path.insert(0, _p)

import bass_rust
import concourse.bass as bass
import concourse.mybir as mybir
import concourse.tile as tile
from concourse.bass_utils import run_bass_kernel_spmd
from concourse.vector_clock import ScopedClock, VectorClock
from contextlib import ExitStack

F32 = mybir.dt.float32
F32R = mybir.dt.float32r
EXP = mybir.ActivationFunctionType.Exp

B = 2
S = 2048
D = 1024
H = 16
HD = 64
NCORES = 8
GROUPS = 4          # head groups (cores per batch)
HG = H // GROUPS    # heads per core = 4
E = HG * HD         # head dims per core = 256
KT = D // 128       # contraction tiles over model dim = 8
SB = S // 128       # s blocks = 16
QB = S // 512       # 512-wide qs blocks = 4

_carrier_counter = [0]


def _split_multi_waits(ordered):
    """This walrus build allows one sync wait per instruction; Tile's wait
    assignment can attach several.  Hoist extras onto same-engine InstNoOp
    carriers placed immediately before the instruction."""
    for bb_name, insts in ordered.items():
        new_list = []
        for inst in insts:
            si = inst.sync_info
            waits = list(si.on_wait) if si is not None else []
            if len(waits) > 1:
                for w in waits[:-1]:
                    _carrier_counter[0] += 1
                    carrier = mybir.InstNoOp(
                        name=f"I-waitc-{_carrier_counter[0]}", ins=[], outs=[]
                    )
                    carrier.engine = inst.engine
                    carrier.sync_info = bass_rust.SyncInfo(on_wait=[w], on_update=[])
                    new_list.append(carrier)
                inst.sync_info = bass_rust.SyncInfo(
                    on_wait=[waits[-1]],
                    on_update=list(si.on_update) if si is not None else [],
                )
            new_list.append(inst)
        ordered[bb_name] = new_list


class _TileContext(tile.TileContext):
    """TileContext adapted to the one-sync-wait-per-instruction walrus."""

    def _lower_ordered_insts(self, ordered):
        _split_multi_waits(ordered)
        return super()._lower_ordered_insts(ordered)

    def _drain_and_barrier(self, tick_clock, wait_clock):
        gc = tick_clock.global_clock
        for proc in range(len(gc)):
            if gc[proc] <= 0:
                continue
            cur = VectorClock([0 if i == proc else gc[i] for i in range(len(gc))])
            nop = self.nc.sync.nop()
            wait_clock.add_sem_waits(
                nop.ins, ScopedClock({None: gc}), ScopedClock({None: cur})
            )
        drain_inst = self.nc.sync.drain()
        wait_clock.add_sem_waits(
            drain_inst.ins, ScopedClock({None: gc}), ScopedClock({None: gc.copy()})
        )
        self.nc.all_engine_barrier()
        assert self.sems is not None
        popped = self.nc._tile_sem_poison_stack.pop()
        assert popped is self._sem_poison
        self.nc.clear_and_free_semaphores(list(self.sems.allocated().values()))
        self.nc.all_engine_barrier()


def build_nc(reps=1):
    nc = bass.Bass()
    xT = nc.declare_dram_parameter("xT", [D, S], F32R, isOutput=False)
    wqT = nc.declare_dram_parameter("wqT", [D, E], F32R, isOutput=False)
    wkT = nc.declare_dram_parameter("wkT", [D, E], F32R, isOutput=False)
    wvT = nc.declare_dram_parameter("wvT", [D, E], F32R, isOutput=False)
    woT = nc.declare_dram_parameter("woT", [E, D], F32R, isOutput=False)
    out = nc.declare_dram_parameter("out_partial", [S, D], F32, isOutput=True)
    ones_d = nc.declare_dram_parameter("ones_d", [128, SB * HG], F32R, isOutput=False)
    den_d = nc.dram_tensor("den_scratch", [HG, S], F32)
    attn_odd_d = nc.dram_tensor("attn_odd_scratch", [2, 64, S], F32R)

    with _TileContext(nc) as tc, ExitStack() as outer:
      for _rep in range(reps):
        ctx = outer.enter_context(ExitStack())
        # ---- persistent activation tiles (live across phases) ----
        act_pool = ctx.enter_context(tc.tile_pool(name="acts", bufs=1))
        qT_sb = [act_pool.tile([128, S], F32R, tag=f"qT{m}", name=f"qT{m}") for m in range(2)]
        kT_sb = [act_pool.tile([128, S], F32R, tag=f"kT{m}", name=f"kT{m}") for m in range(2)]
        v_sb = act_pool.tile([128, SB, HG, HD + 1], F32R, tag="v")
        wo_sb = [act_pool.tile([128, D], F32R, tag=f"wo{m}", name=f"wo{m}") for m in range(2)]

        # ---- phase 1: load inputs, project q/k/v ----
        # x/wq/wk outlive c1: the m=1 head-pair's q/k projections are emitted
        # after the first attention group (borrowing its PV psum slots) so
        # they fill PE slack instead of delaying the first exps
        c1x = ctx.enter_context(ExitStack())
        xqk_pool = c1x.enter_context(tc.tile_pool(name="xqk", bufs=1, side="right"))
        x_sb = xqk_pool.tile([128, KT, S], F32R, tag="x")
        wq_sb = xqk_pool.tile([128, KT, E], F32R, tag="wq")
        wk_sb = xqk_pool.tile([128, KT, E], F32R, tag="wk")
        with ExitStack() as c1:
            in_pool = c1.enter_context(tc.tile_pool(name="ins", bufs=1))
            ps_qk = c1.enter_context(tc.tile_pool(name="ps_qk", bufs=2, space="PSUM"))
            ps_v = c1.enter_context(tc.tile_pool(name="ps_v", bufs=2, space="PSUM"))

            wv_sb = in_pool.tile([128, KT, E], F32R, tag="wv")
            # weights first (small), then x in 256KB qs-major chunks split
            # across both HWDGE queues — the first qT psum group needs only
            # wq + the first 8 x chunks instead of the whole 8MB of x
            for k in range(KT):
                eng = nc.sync if k % 2 == 0 else nc.gpsimd
                eng2 = nc.gpsimd if k % 2 == 0 else nc.sync
                eng.dma_start(wq_sb[:, k, :], wqT[k * 128:(k + 1) * 128, :])
                eng2.dma_start(wk_sb[:, k, :], wkT[k * 128:(k + 1) * 128, :])
                eng.dma_start(wv_sb[:, k, :], wvT[k * 128:(k + 1) * 128, :])
            for nb in range(QB):
                for k in range(KT):
                    eng = nc.sync if k % 2 == 0 else nc.gpsimd
                    eng.dma_start(
                        x_sb[:, k, nb * 512:(nb + 1) * 512],
                        xT[k * 128:(k + 1) * 128, nb * 512:(nb + 1) * 512],
                    )
            for m in range(2):
                nc.gpsimd.dma_start(wo_sb[m][:, :], woT[m * 128:(m + 1) * 128, :])

            # ones column for the softmax-denominator rows of v
            nc.sync.dma_start(
                v_sb[:, :, :, HD],
                ones_d[:, :].rearrange("p (s h) -> p s h", s=SB),
            )

            # qT / kT: W-stationary, out [e(128), s]; qs-major so compute
            # starts as soon as the first x column-block lands
            for nb in range(QB):
                for w_sb, dst in ((wq_sb, qT_sb), (wk_sb, kT_sb)):
                    for m in (0,):
                        ps = ps_qk.tile([128, 512], F32)
                        for k in range(KT):
                            nc.tensor.matmul(
                                ps[:],
                                w_sb[:, k, m * 128:(m + 1) * 128],
                                x_sb[:, k, nb * 512:(nb + 1) * 512],
                                start=(k == 0),
                                stop=(k == KT - 1),
                            )
                        # alternate copies between DVE and ACT (ACT idle here)
                        if (nb + m) % 2 == 0:
                            nc.vector.tensor_copy(
                                dst[m][:, nb * 512:(nb + 1) * 512], ps[:]
                            )
                        else:
                            nc.scalar.copy(
                                dst[m][:, nb * 512:(nb + 1) * 512], ps[:]
                            )
            # v: x-stationary, out [s(128), e]
            for sb in range(SB):
                ps = ps_v.tile([128, E], F32)
                for k in range(KT):
                    nc.tensor.matmul(
                        ps[:],
                        x_sb[:, k, sb * 128:(sb + 1) * 128],
                        wv_sb[:, k, :],
                        start=(k == 0),
                        stop=(k == KT - 1),
                    )
                if sb % 2 == 0:
                    nc.vector.tensor_copy(
                        v_sb[:, sb, :, 0:HD],
                        ps[:].rearrange("p (h e) -> p h e", h=HG),
                    )
                else:
                    nc.scalar.copy(
                        v_sb[:, sb, :, 0:HD],
                        ps[:].rearrange("p (h e) -> p h e", h=HG),
                    )

        # ---- phase 2: attention per head-pair (m), per qs-half ----
        # attn lands in paired [128, S] tiles (head 2m rows 0-63, head 2m+1
        # rows 64-127) so the projection contracts K=128 per pair.  Odd heads
        # can't be written to partitions 64-127 by compute engines, so they
        # stage at partitions 0-63 and round-trip through DRAM (DMA shifts
        # partitions freely).
        attn_pool = ctx.enter_context(tc.tile_pool(name="attn", bufs=1))
        attn_pair = [attn_pool.tile([128, S], F32R, tag=f"ap{m}", name=f"ap{m}") for m in range(2)]
        attn_odd = [attn_pool.tile([64, S], F32R, tag=f"ao{m}", name=f"ao{m}") for m in range(2)]
        with ExitStack() as c2:
            p_pool = c2.enter_context(tc.tile_pool(name="pexp", bufs=4))
            rb_pool = c2.enter_context(tc.tile_pool(name="rbc", bufs=2))
            den_pool = c2.enter_context(tc.tile_pool(name="den", bufs=2))
            ps_pv = c2.enter_context(tc.tile_pool(name="ps_pv", bufs=1, space="PSUM"))
            ps_sc = c2.enter_context(tc.tile_pool(name="ps_sc", bufs=1, space="PSUM"))

            for m in range(2):
                for qh in range(2):  # qs halves of 1024
                    if m == 0 and qh == 1:
                        # second head-pair q/k projections, on PV psum slots
                        gi = 0
                        for nb in range(QB):
                            for w_sb, dst in ((wq_sb, qT_sb), (wk_sb, kT_sb)):
                                ps = ps_pv.tile(
                                    [128, 512], F32,
                                    tag=f"pv{gi % 2}{(gi // 2) % 2}",
                                    name=f"ph1b{gi}",
                                )
                                gi += 1
                                for k in range(KT):
                                    nc.tensor.matmul(
                                        ps[:],
                                        w_sb[:, k, 128:256],
                                        x_sb[:, k, nb * 512:(nb + 1) * 512],
                                        start=(k == 0),
                                        stop=(k == KT - 1),
                                    )
                                nc.vector.tensor_copy(
                                    dst[1][:, nb * 512:(nb + 1) * 512], ps[:]
                                )
                        c1x.close()
                    out_ps = [
                        [ps_pv.tile([128, 512], F32, tag=f"pv{r}{qq}", name=f"pv{r}{qq}") for qq in range(2)]
                        for r in range(2)
                    ]
                    for ksb in range(SB):
                        sc = [ps_sc.tile([128, 1024], F32, tag=f"sc{r}", name=f"sc{r}") for r in range(2)]
                        for qq in range(2):
                            for r in range(2):
                                nc.tensor.matmul(
                                    sc[r][:, qq * 512:(qq + 1) * 512],
                                    kT_sb[m][64 * r:64 * r + 64,
                                             ksb * 128:(ksb + 1) * 128],
                                    qT_sb[m][64 * r:64 * r + 64,
                                             qh * 1024 + qq * 512:
                                             qh * 1024 + (qq + 1) * 512],
                                    start=True,
                                    stop=True,
                                )
                        pt = [None, None]
                        for r in range(2):
                            pt[r] = p_pool.tile([128, 1024], F32R, tag="p", name=f"p{r}")
                            nc.scalar.activation(pt[r][:], sc[r][:], EXP)
                        for r in range(2):
                            for qq in range(2):
                                nc.tensor.matmul(
                                    out_ps[r][qq][0:HD + 1, :],
                                    v_sb[:, ksb, 2 * m + r, :],
                                    pt[r][:, qq * 512:(qq + 1) * 512],
                                    start=(ksb == 0),
                                    stop=(ksb == SB - 1),
                                )
                    # normalize: attn = attnU * (1/denom), denom = row 64.
                    # Copy attnU + denom out of PSUM immediately (frees the
                    # accumulation banks for the next group); the broadcasted
                    # reciprocal multiply happens later in SBUF, off the
                    # PSUM critical path.
                    qsl = slice(qh * 1024, (qh + 1) * 1024)
                    rb = rb_pool.tile([128, 1024], F32, tag="rb")
                    for r in range(2):
                        h = 2 * m + r
                        # even head -> pair tile rows 0-63 directly; odd head
                        # -> staging tile, round-tripped UNNORMALIZED through
                        # DRAM to pair rows 64-127 so the shift overlaps the
                        # reciprocal/broadcast chain instead of following it
                        dst = attn_pair[m] if r == 0 else attn_odd[m]
                        den = den_pool.tile([128, 1024], F32, tag="den")
                        last_group = (m == 1 and qh == 1)
                        for qq in range(2):
                            qs0 = qh * 1024 + qq * 512
                            nc.vector.tensor_copy(
                                den[64:65, qq * 512:(qq + 1) * 512],
                                out_ps[r][qq][64:65, :],
                            )
                            if last_group and qq == 1:
                                # ACT is idle at the phase-2 tail; split the
                                # bank-releasing copies so the projection
                                # phase starts sooner
                                nc.scalar.copy(
                                    dst[0:64, qs0:qs0 + 512],
                                    out_ps[r][qq][0:64, :],
                                )
                            else:
                                nc.vector.tensor_copy(
                                    dst[0:64, qs0:qs0 + 512],
                                    out_ps[r][qq][0:64, :],
                                )
                        if r == 1:
                            nc.sync.dma_start(
                                attn_odd_d[m, :, qsl], attn_odd[m][0:64, qsl]
                            )
                            nc.gpsimd.dma_start(
                                attn_pair[m][64:128, qsl], attn_odd_d[m, :, qsl]
                            )
                        nc.vector.reciprocal(den[64:65, :], den[64:65, :])
                        nc.sync.dma_start(
                            den_d[h:h + 1, qh * 1024:(qh + 1) * 1024], den[64:65, :]
                        )
                        # broadcast 1/denom straight to this head's partition
                        # range of the shared rb tile
                        nc.sync.dma_start(
                            rb[64 * r:64 * r + 64, :],
                            den_d[h:h + 1, qh * 1024:(qh + 1) * 1024]
                            .to_broadcast((64, 1024)),
                        )
                    for r in range(2):
                        p0 = 64 * r
                        for qq in range(2):
                            qs0 = qh * 1024 + qq * 512
                            nc.vector.tensor_mul(
                                attn_pair[m][p0:p0 + 64, qs0:qs0 + 512],
                                attn_pair[m][p0:p0 + 64, qs0:qs0 + 512],
                                rb[p0:p0 + 64, qq * 512:(qq + 1) * 512],
                            )

        # ---- phase 3: output projection (row-parallel partial) ----
        with ExitStack() as c3:
            stage_pool = c3.enter_context(tc.tile_pool(name="ostage", bufs=6))
            ps_o = c3.enter_context(
                tc.tile_pool(name="ps_o", bufs=4, space="PSUM")
            )
            for sb in range(SB):
                for nb in range(2):
                    ps = ps_o.tile([128, 512], F32)
                    for m in range(2):
                        nc.tensor.matmul(
                            ps[:],
                            attn_pair[m][:, sb * 128:(sb + 1) * 128],
                            wo_sb[m][:, nb * 512:(nb + 1) * 512],
                            start=(m == 0),
                            stop=(m == 1),
                        )
                    st = stage_pool.tile([128, 512], F32, tag="st")
                    if (2 * sb + nb) % 2 == 0:
                        nc.vector.tensor_copy(st[:], ps[:])
                    else:
                        nc.scalar.copy(st[:], ps[:])
                    eng = nc.sync if nb == 0 else nc.gpsimd
                    eng.dma_start(
                        out[sb * 128:(sb + 1) * 128, nb * 512:(nb + 1) * 512],
                        st[:],
                    )
        ctx.close()
    return nc


_NC_CACHE = None


def _get_nc():
    global _NC_CACHE
    if _NC_CACHE is None:
        _NC_CACHE = build_nc()
    return _NC_CACHE


_EXEC_CACHE = None


def _get_executor():
    """Build + jit the SPMD executable once; reuse across kernel() calls.

    Mirrors concourse.bass2jax.run_bass_via_pjrt, which re-jits on every
    call (full retrace + executable reload); caching shaves seconds/call."""
    global _EXEC_CACHE
    if _EXEC_CACHE is not None:
        return _EXEC_CACHE
    import jax
    from jax.sharding import Mesh, PartitionSpec
    from jax.experimental.shard_map import shard_map
    from concourse import bass2jax as b2j

    nc = _get_nc()
    b2j.install_neuronx_cc_hook()
    assert nc.dbg_addr is None
    partition_name = (
        nc.partition_id_tensor.name if nc.partition_id_tensor is not None else None
    )

    in_names, out_names, out_avals = [], [], []
    for alloc in nc.m.functions[0].allocations:
        if not isinstance(alloc, mybir.MemoryLocationSet):
            continue
        name = alloc.memorylocations[0].name
        if alloc.kind == "ExternalInput":
            if name != partition_name:
                in_names.append(name)
        elif alloc.kind == "ExternalOutput":
            out_names.append(name)
            out_avals.append(
                jax.core.ShapedArray(
                    tuple(alloc.tensor_shape), mybir.dt.np(alloc.dtype)
                )
            )
    n_params = len(in_names)
    n_outs = len(out_avals)
    all_names = in_names + out_names
    if partition_name is not None:
        all_names = all_names + [partition_name]

    def _body(*args):
        operands = list(args)
        if partition_name is not None:
            operands.append(b2j.partition_id_tensor())
        outs = b2j._bass_exec_p.bind(
            *operands,
            out_avals=tuple(out_avals),
            in_names=tuple(all_names),
            out_names=tuple(out_names),
            lowering_input_output_aliases=(),
            sim_require_finite=True,
            sim_require_nnan=True,
            nc=nc,
        )
        return tuple(outs)

    devices = jax.devices()[:NCORES]
    mesh = Mesh(np.asarray(devices), ("core",))
    donate = tuple(range(n_params, n_params + n_outs))
    sharded = jax.jit(
        shard_map(
            _body,
            mesh=mesh,
            in_specs=(PartitionSpec("core"),) * (n_params + n_outs),
            out_specs=(PartitionSpec("core"),) * n_outs,
            check_rep=False,
        ),
        donate_argnums=donate,
        keep_unused=True,
    )
    import jax.numpy as jnp

    zero_shardings = [
        jax.sharding.NamedSharding(mesh, PartitionSpec("core"))
    ] * n_outs

    @jax.jit
    def _make_zeros():
        return tuple(
            jax.lax.with_sharding_constraint(
                jnp.zeros((NCORES * a.shape[0], *a.shape[1:]), a.dtype), sh
            )
            for a, sh in zip(out_avals, zero_shardings)
        )

    _EXEC_CACHE = {
        "sharded": sharded,
        "make_zeros": _make_zeros,
        "in_names": in_names,
        "out_names": out_names,
        "out_avals": out_avals,
    }
    return _EXEC_CACHE


def _run_spmd(in_maps):
    ex = _get_executor()
    concat_in = [
        np.concatenate([np.asarray(m[name]) for m in in_maps], axis=0)
        for name in ex["in_names"]
    ]
    concat_zeros = ex["make_zeros"]()
    out_arrs = ex["sharded"](*concat_in, *concat_zeros)
    results = []
    for c in range(NCORES):
        results.append({
            name: np.asarray(out_arrs[i]).reshape(
                NCORES, *ex["out_avals"][i].shape
            )[c]
            for i, name in enumerate(ex["out_names"])
        })
    return results


def _shard_inputs(x, Wq, Wk, Wv, Wo):
    scale = np.float32(1.0 / np.sqrt(HD))
    global _ONES
    _ONES = np.ones((128, SB * HG), dtype=np.float32)
    in_maps = []
    xT_b = [np.ascontiguousarray(x[b].T) for b in range(B)]
    for c in range(NCORES):
        b, g = divmod(c, GROUPS)
        sl = slice(g * E, (g + 1) * E)
        in_maps.append({
            "ones_d": _ONES,
            "xT": xT_b[b],
            "wqT": np.ascontiguousarray(Wq[sl, :].T * scale),
            "wkT": np.ascontiguousarray(Wk[sl, :].T),
            "wvT": np.ascontiguousarray(Wv[sl, :].T),
            "woT": np.ascontiguousarray(Wo[:, sl].T),
        })
    return in_maps


_FAST_PATH_OK = True


def kernel(x, Wq, Wk, Wv, Wo, bo):
    global _FAST_PATH_OK
    x = np.asarray(x, dtype=np.float32)
    in_maps = _shard_inputs(
        x,
        np.asarray(Wq, dtype=np.float32),
        np.asarray(Wk, dtype=np.float32),
        np.asarray(Wv, dtype=np.float32),
        np.asarray(Wo, dtype=np.float32),
    )
    results = None
    if _FAST_PATH_OK:
        try:
            results = _run_spmd(in_maps)
        except Exception:
            _FAST_PATH_OK = False
    if results is None:
        # portable fallback: stock SPMD runner (handles native-device
        # environments and anything the cached-PJRT fast path can't)
        results = run_bass_kernel_spmd(
            _get_nc(), in_maps, list(range(NCORES))
        ).results
    bo = np.asarray(bo, dtype=np.float32)
    out = np.empty((B, S, D), dtype=np.float32)
    for b in range(B):
        acc = np.zeros((S, D), dtype=np.float64)
        for g in range(GROUPS):
            acc += results[b * GROUPS + g]["out_partial"]
        out[b] = (acc + bo.astype(np.float64)).astype(np.float32)
    return out
